# revision 1
# baseline (speedup 1.0000x reference)
"""Causal self-attention (B=1, T=4096, C=768, H=12) on 8 Trainium2 NeuronCores.

Sharding: tensor-parallel over heads. 16 head-slots across 8 cores (2 slots
per core); 12 real heads + 4 dummy slots with zeroed weights. Each core:
  1. transposes x -> x^T on the PE array (needed as matmul contraction layout)
  2. projects Q^T, K^T, V^T for its 2 head-slots (full T)
  3. runs causal flash-style attention fully on-chip in the transposed
     layout: S^T[k,q] = K^T.T @ Q^T per 128-wide k-block, P^T = exp(S^T/8)
     (scores are small enough that max-subtraction is unnecessary), causal
     masking via static triangular masks on the diagonal blocks only, and
     y^T accumulated in PSUM with an extra all-ones column in V providing
     the softmax denominator in row 64.
  4. normalizes y^T and computes a partial output projection with its
     128-row slice of w_proj.
The host sums the 8 partial [4096, 768] outputs -- no device collectives.

Causal load within a core is balanced by processing q-blocks in pairs
(i, 15-i) of 256 rows: each pair touches exactly 34 k-blocks.
"""

import sys

sys.path.insert(0, "/opt/trn_rl_repo")

import numpy as np

T = 4096
C = 768
H = 12
HD = 64
N_CORES = 8
SLOTS = 2
TS = 512  # t-slice for x load/transpose/projection
NTS = T // TS  # 8
QB = 256  # q-block rows
NQB = T // QB  # 16
KB = 128  # k-block rows
NKB = T // KB  # 32
NPAIR = NQB // 2  # 8 causal-balanced pairs (i, 15-i)

_CACHE = {}


def _paired_col(b256: int) -> int:
    """Column offset of 256-row q-block b256 in the paired SBUF layout.

    Pair p = min(b, 15-b) occupies cols [512p, 512p+512): side A (b < 8)
    at +0, side B (b >= 8) at +256.
    """
    p = min(b256, NQB - 1 - b256)
    side = 1 if b256 >= NQB // 2 else 0
    return 2 * QB * p + QB * side


def _build_nc():
    import concourse.bacc as bacc
    import concourse.tile as tile
    import concourse.mybir as mybir
    from concourse.masks import make_identity
    from contextlib import ExitStack

    F32 = mybir.dt.float32
    F32R = mybir.dt.float32r
    EXP = mybir.ActivationFunctionType.Exp

    nc = bacc.Bacc(
        "TRN2",
        target_bir_lowering=False,
        debug=False,
        enable_asserts=True,
        num_devices=N_CORES,
    )
    x_d = nc.dram_tensor("x", [T, C], F32R, kind="ExternalInput")
    wa_d = nc.dram_tensor("wa", [C, 3 * SLOTS * HD], F32R, kind="ExternalInput")
    wp_d = nc.dram_tensor("wp", [SLOTS * HD, C], F32R, kind="ExternalInput")
    out_d = nc.dram_tensor("out", [T, C], F32, kind="ExternalOutput")

    with ExitStack() as ctx:
        tc = ctx.enter_context(tile.TileContext(nc))
        singles = ctx.enter_context(tc.tile_pool(name="singles", bufs=1))
        xpool = ctx.enter_context(tc.tile_pool(name="xpool", bufs=8))
        xtpool = ctx.enter_context(tc.tile_pool(name="xtpool", bufs=8))
        ptpool = ctx.enter_context(tc.tile_pool(name="ptpool", bufs=8))
        rpool = ctx.enter_context(tc.tile_pool(name="rpool", bufs=4))
        opool = ctx.enter_context(tc.tile_pool(name="opool", bufs=4))
        ps = ctx.enter_context(tc.tile_pool(name="ps", bufs=2, space="PSUM"))
        ps_st = ctx.enter_context(tc.tile_pool(name="ps_st", bufs=2, space="PSUM"))
        ps_yt = ctx.enter_context(tc.tile_pool(name="ps_yt", bufs=2, space="PSUM"))

        # ---- persistent SBUF tensors ----
        qt = singles.tile([128, T], F32R)  # Q^T, paired column layout
        kt = singles.tile([128, T], F32R)  # K^T, natural column layout
        yt_all = singles.tile([128, T], F32R)  # normalized y^T, paired layout
        v1 = singles.tile([128, NKB, SLOTS, HD + 1], F32R)  # V blocks + ones col
        wa_sb = []
        for i in range(6):
            wa_c = singles.tile([128, 3 * SLOTS * HD], F32R, name=f"wa_c{i}")
            wa_sb.append(wa_c)
        wp_sb = singles.tile([SLOTS * HD, C], F32R)
        ident = singles.tile([128, 128], F32R)
        ones64 = singles.tile([1, HD], F32R)
        maskf = singles.tile([128, 3 * KB], F32)  # [:,128:384]=M0, [:,0:256]=M1

        ident_f32 = singles.tile([128, 128], F32)
        make_identity(nc, ident_f32)
        nc.vector.tensor_copy(out=ident, in_=ident_f32)
        ones_f32 = singles.tile([128, NKB * SLOTS], F32)
        nc.gpsimd.memset(ones_f32, 1.0)
        nc.vector.tensor_copy(out=ones64, in_=ones_f32[0:1, 0:HD])
        nc.vector.tensor_copy(
            out=v1[:, :, :, HD : HD + 1],
            in_=ones_f32.rearrange("p (a b) -> p a b", a=NKB).unsqueeze(3),
        )

        # maskf[k, c] = 1 if c >= k + 128 else 0
        nc.gpsimd.memset(maskf, 0.0)
        nc.gpsimd.affine_select(
            out=maskf,
            in_=maskf,
            compare_op=mybir.AluOpType.is_gt,
            fill=1.0,
            base=KB,
            channel_multiplier=1,
            pattern=[[-1, 3 * KB]],
        )

        for i in range(6):
            nc.gpsimd.dma_start(out=wa_sb[i], in_=wa_d.ap()[i * 128 : (i + 1) * 128, :])
        nc.gpsimd.dma_start(out=wp_sb, in_=wp_d.ap())

        # ---- phase A/B: x -> x^T -> Q^T/K^T/V per t-slice ----
        for ts in range(NTS):
            xs = []
            for tb in range(4):
                r0 = ts * TS + tb * 128
                xst = xpool.tile([128, C], F32R, name="xs", tag="xs")
                nc.sync.dma_start(out=xst, in_=x_d.ap()[r0 : r0 + 128, :])
                xs.append(xst)
            xts = []
            for ic in range(6):
                xtt = xtpool.tile([128, TS], F32R, name="xt", tag="xt")
                tps = ps.tile([128, TS], F32R, name="tps", tag="ps")
                for tb in range(4):
                    nc.tensor.transpose(
                        tps[:, tb * 128 : (tb + 1) * 128],
                        xs[tb][:, ic * 128 : (ic + 1) * 128],
                        ident,
                    )
                nc.vector.tensor_copy(out=xtt, in_=tps)
                xts.append(xtt)
            for p in range(3):
                pp = ps.tile([128, TS], F32, name="pp", tag="ps")
                for ic in range(6):
                    nc.tensor.matmul(
                        pp,
                        lhsT=wa_sb[ic][:, p * 128 : (p + 1) * 128],
                        rhs=xts[ic],
                        start=(ic == 0),
                        stop=(ic == 5),
                    )
                if p == 0:
                    for half in range(2):
                        col = _paired_col(2 * ts + half)
                        nc.vector.tensor_copy(
                            out=qt[:, col : col + QB],
                            in_=pp[:, half * QB : (half + 1) * QB],
                        )
                elif p == 1:
                    nc.vector.tensor_copy(out=kt[:, ts * TS : (ts + 1) * TS], in_=pp)
                else:
                    vt = rpool.tile([128, TS], F32R, name="vt", tag="vt", bufs=3)
                    nc.vector.tensor_copy(out=vt, in_=pp)
                    for sub in range(4):
                        kb = 4 * ts + sub
                        vps = ps.tile([128, 128], F32R, name="vps", tag="ps")
                        nc.tensor.transpose(
                            vps,
                            vt[:, sub * 128 : (sub + 1) * 128],
                            ident,
                        )
                        nc.vector.tensor_copy(
                            out=v1[:, kb, :, 0:HD],
                            in_=vps.rearrange("p (s d) -> p s d", s=SLOTS),
                        )

        # ---- phase C: attention, slots interleaved per pair; phase D
        # (partial projection) emitted as soon as a pair completes ----
        scale = 1.0 / float(np.sqrt(HD))

        def emit_proj(tb):
            b256, half = tb // 2, tb % 2
            col = _paired_col(b256) + 128 * half
            po = ps_st.tile([128, C], F32, name="po", tag="st")
            for c0, c1 in ((0, 512), (512, 768)):  # bank-aligned splits
                nc.tensor.matmul(
                    po[:, c0:c1],
                    lhsT=yt_all[:, col : col + 128],
                    rhs=wp_sb[:, c0:c1],
                    start=True,
                    stop=True,
                )
            osb = opool.tile([128, C], F32, name="osb", tag="osb")
            nc.vector.tensor_copy(out=osb, in_=po)
            nc.sync.dma_start(
                out=out_d.ap()[tb * 128 : (tb + 1) * 128, :], in_=osb
            )

        import collections
        work_q = collections.deque()  # deferred closures, drained between groups

        def emit_norm(ytsb, r0, r1, col):
            def go():
                r_sb = rpool.tile([1, 2 * QB], F32R, name="r_sb", tag="r_sb", bufs=8)
                with nc.allow_low_precision(reason="fp32r softmax denom"):
                    nc.vector.reciprocal(out=r_sb, in_=ytsb[HD : HD + 1, :])
                bc = ps.tile([HD, 2 * QB], F32, name="bc", tag="ps")
                nc.tensor.matmul(
                    bc,
                    lhsT=ones64,
                    rhs=r_sb,
                    start=True,
                    stop=True,
                )
                nc.vector.tensor_mul(
                    out=yt_all[r0:r1, col : col + 2 * QB],
                    in0=ytsb[0:HD, :],
                    in1=bc,
                )
            return go

        for i in reversed(range(NPAIR)):
            for s in range(SLOTS):
                r0, r1 = s * HD, (s + 1) * HD
                qcol = 2 * QB * i
                n_shared = 2 * i + 2  # k-blocks needed by side A (block i)
                n_total = NKB - 2 * i  # k-blocks needed by side B (block 15-i)
                diag_b0 = NKB - 2 - 2 * i  # first diagonal k-block of side B
                yt = ps_yt.tile([HD + 1, 2 * QB], F32, name="yt", tag="yt")
                # k-blocks in groups sharing one wide PSUM score tile:
                # shared region (A+B, q-width 512) pairs 2 k-blocks; solo
                # region (B only, q-width 256) packs 4. One exp per group.
                groups = [list(range(g, g + 2)) for g in range(0, n_shared, 2)]
                kb0 = n_shared
                while kb0 < n_total:
                    n = min(4, n_total - kb0)
                    groups.append(list(range(kb0, kb0 + n)))
                    kb0 += n
                def emit_s(grp):
                    shared = grp[0] < n_shared
                    w = 2 * QB if shared else QB
                    qoff = qcol if shared else qcol + QB
                    gw = w * len(grp)
                    st = ps_st.tile([128, 4 * QB], F32, name="st", tag="st")
                    for j, kb in enumerate(grp):
                        nc.tensor.matmul(
                            st[:, j * w : (j + 1) * w],
                            lhsT=kt[r0:r1, kb * KB : (kb + 1) * KB],
                            rhs=qt[r0:r1, qoff : qoff + w],
                            start=True,
                            stop=True,
                        )
                    pt = ptpool.tile([128, 4 * QB], F32R, name="pt", tag="pt")
                    nc.scalar.activation(
                        out=pt[:, 0:gw], in_=st[:, 0:gw], func=EXP, scale=scale
                    )
                    for j, kb in enumerate(grp):
                        pA = pt[:, j * w : j * w + QB]
                        if kb == 2 * i or kb == diag_b0:
                            nc.vector.tensor_mul(
                                out=pA, in0=pA, in1=maskf[:, KB : KB + QB]
                            )
                        if kb == 2 * i + 1 or kb == diag_b0 + 1:
                            nc.vector.tensor_mul(
                                out=pA, in0=pA, in1=maskf[:, 0:QB]
                            )
                    return pt, w

                def emit_pv(grp, pt, w):
                    # one PSUM accumulation group spans the whole pair:
                    # started once at kb==0 (full width), A columns simply
                    # stop being written after the shared region ends,
                    # stop flags on the final solo matmul
                    shared = grp[0] < n_shared
                    for j, kb in enumerate(grp):
                        vblk = v1[:, kb, s, :]
                        if shared:
                            nc.tensor.matmul(
                                yt,
                                lhsT=vblk,
                                rhs=pt[:, j * w : (j + 1) * w],
                                start=(kb == 0),
                                stop=False,
                                skip_group_check=True,
                            )
                        else:
                            nc.tensor.matmul(
                                yt[:, QB : 2 * QB],
                                lhsT=vblk,
                                rhs=pt[:, j * w : (j + 1) * w],
                                start=False,
                                stop=(kb == n_total - 1),
                                skip_group_check=True,
                            )

                pending = None
                for grp in groups:
                    cur = (grp, *emit_s(grp))
                    if pending is not None:
                        emit_pv(*pending)
                    pending = cur
                    if work_q:
                        work_q.popleft()()
                emit_pv(*pending)
                # free the yt PSUM slot immediately; queue the rest of
                # the normalization to drain between later matmul groups
                ytsb = rpool.tile([HD + 1, 2 * QB], F32, name="ytsb", tag="ytsb", bufs=6)
                nc.vector.tensor_copy(out=ytsb, in_=yt)
                work_q.append(emit_norm(ytsb, r0, r1, qcol))
            for tb in (2 * i, 2 * i + 1, NKB - 2 - 2 * i, NKB - 1 - 2 * i):
                work_q.append(lambda tb=tb: emit_proj(tb))
        while work_q:
            work_q.popleft()()


    nc.compile()
    return nc


def _get_nc():
    if "nc" not in _CACHE:
        _CACHE["nc"] = _build_nc()
    return _CACHE["nc"]


def _core_inputs(x, w_attn, w_proj):
    """Build per-core input dicts (head-slot weight slices)."""
    x = np.ascontiguousarray(x.reshape(T, C), dtype=np.float32)
    w_attn = np.asarray(w_attn, dtype=np.float32)
    w_proj = np.asarray(w_proj, dtype=np.float32)
    in_maps = []
    for c in range(N_CORES):
        heads = [c, 8 + c if c < 4 else None]
        wa = np.zeros((C, 3, SLOTS, HD), dtype=np.float32)
        wp = np.zeros((SLOTS * HD, C), dtype=np.float32)
        for s, h in enumerate(heads):
            if h is None:
                continue
            for p in range(3):
                wa[:, p, s, :] = w_attn[:, p * C + h * HD : p * C + (h + 1) * HD]
            wp[s * HD : (s + 1) * HD, :] = w_proj[h * HD : (h + 1) * HD, :]
        in_maps.append(
            {"x": x, "wa": np.ascontiguousarray(wa.reshape(C, 3 * SLOTS * HD)), "wp": wp}
        )
    return in_maps


def _get_runner():
    """Build the shard_map'd PJRT executable once and reuse it across calls.

    Mirrors bass2jax.run_bass_via_pjrt's multi-core path, but caches the
    jitted callable so repeat kernel() calls skip re-trace/re-compile.
    """
    if "runner" in _CACHE:
        return _CACHE["runner"]
    import jax
    import concourse.mybir as mybir
    from concourse import bass2jax
    from jax.experimental.shard_map import shard_map
    from jax.sharding import Mesh, PartitionSpec

    nc = _get_nc()
    bass2jax.install_neuronx_cc_hook()

    in_names, out_names, out_avals, zero_outs = [], [], [], []
    for alloc in nc.m.functions[0].allocations:
        if not isinstance(alloc, mybir.MemoryLocationSet):
            continue
        name = alloc.memorylocations[0].name
        if alloc.kind == "ExternalInput":
            if nc.partition_id_tensor and name == nc.partition_id_tensor.name:
                continue
            in_names.append(name)
        elif alloc.kind == "ExternalOutput":
            shape = tuple(alloc.tensor_shape)
            dtype = mybir.dt.np(alloc.dtype)
            out_names.append(name)
            out_avals.append(jax.core.ShapedArray(shape, dtype))
            zero_outs.append(np.zeros(shape, dtype))
    n_params = len(in_names)
    all_in_names = in_names + out_names
    if nc.partition_id_tensor:
        all_in_names = all_in_names + [nc.partition_id_tensor.name]

    def _body(*args):
        operands = list(args)
        if nc.partition_id_tensor:
            operands.append(bass2jax.partition_id_tensor())
        outs = bass2jax._bass_exec_p.bind(
            *operands,
            out_avals=tuple(out_avals),
            in_names=tuple(all_in_names),
            out_names=tuple(out_names),
            lowering_input_output_aliases=(),
            sim_require_finite=True,
            sim_require_nnan=True,
            nc=nc,
        )
        return tuple(outs)

    devices = jax.devices()[:N_CORES]
    mesh = Mesh(np.asarray(devices), ("core",))
    n_out = len(out_names)
    donate = tuple(range(n_params, n_params + n_out))
    sharded = jax.jit(
        shard_map(
            _body,
            mesh=mesh,
            in_specs=(PartitionSpec("core"),) * (n_params + n_out),
            out_specs=(PartitionSpec("core"),) * n_out,
            check_rep=False,
        ),
        donate_argnums=donate,
        keep_unused=True,
    )

    def run(in_maps):
        concat_in = [
            np.concatenate([in_maps[c][name] for c in range(N_CORES)], axis=0)
            for name in in_names
        ]
        concat_zeros = [
            np.zeros((N_CORES * z.shape[0], *z.shape[1:]), z.dtype)
            for z in zero_outs
        ]
        out_arrs = sharded(*concat_in, *concat_zeros)
        return [
            {
                name: np.asarray(out_arrs[i]).reshape(
                    N_CORES, *out_avals[i].shape
                )[c]
                for i, name in enumerate(out_names)
            }
            for c in range(N_CORES)
        ]

    _CACHE["runner"] = run
    return run


def kernel(x, w_attn, w_proj):
    run = _get_runner()
    in_maps = _core_inputs(np.asarray(x), np.asarray(w_attn), np.asarray(w_proj))
    results = run(in_maps)
    out = np.zeros((T, C), dtype=np.float32)
    for c in range(N_CORES):
        out += results[c]["out"]
    return out.reshape(1, T, C)



# revision 40
# speedup vs baseline: 1.3863x; 1.3863x over previous
"""Causal self-attention (B=1, T=4096, C=768, H=12) on 8 Trainium2 NeuronCores.

Sharding: 24 units = (head, query-half).  Each core owns one full head
(slot 0, all 8 causal-balanced query pairs) plus one half head (slot 1,
4 pairs) -- 1.5 heads of attention area per core, no dummy slots.
Cores 0-3 run a program variant whose slot 1 covers pairs 0-3; cores
4-7 run the complementary variant (pairs 4-7).  The two variants have
identical instruction streams (only column constants differ), so their
simulated/hardware cost is identical.

Head map: core c in 0..3: slot0 = head c, slot1 = head 8+c (pairs 0-3)
          core c in 4..7: slot0 = head c, slot1 = head 4+c (pairs 4-7)

Per core:
  1. x^T arrives pre-transposed and pre-cast to bf16 by the host (a
     sharding-layout choice; no PE transposes anywhere).  Q^T/K^T are
     projected per 512-col t-slice into [128, T] (2 heads x 64 dims on
     partitions); V is projected directly in natural [t, d] layout and
     cast to fp8e4 with an appended ones column that accumulates the
     softmax denominator during P@V.  QKV work is interleaved into the
     attention stream as fillers so the Act engine starts early.
  2. Flash-style causal attention, q-blocks in balanced pairs (i, 15-i):
     S^T = K^T.T @ Q^T per 128-wide k-block (bf16).  Causal masking
     pre-writes a -1e30 bias tile into PSUM and lets the diagonal S
     matmuls accumulate onto it (start=False), keeping the mask off the
     S->exp critical chain.  P = exp(S^T/8) on the Act engine writes
     fp8 directly; P@V runs in fp8 DoubleRow mode (two k-blocks
     contracted per matmul at 0.5 cycles/row) accumulating y^T plus
     denominator in PSUM.
  3. y^T normalized via reciprocal + ones-broadcast matmul; partial
     output projection (bank-aligned 512/256 chunks through the shared
     1-bank PSUM pool); partial sums DMA'd out in bf16 and summed on
     the host.  Norm/proj run as deferred closures drained a full pair
     behind the attention front so their dependencies are always ready
     (the PE wait-queue is strict in-order; a stalled matmul blocks
     everything behind it).
"""

import sys

sys.path.insert(0, "/opt/trn_rl_repo")

import numpy as np

T = 4096
C = 768
H = 12
HD = 64
N_CORES = 8
QB = 256  # q-block rows
NQB = T // QB  # 16
KB = 128  # k-block rows
NKB = T // KB  # 32
NPAIR = NQB // 2  # 8 causal-balanced pairs (i, 15-i)
TS = 512  # t-slice for Q/K projection
NTS = T // TS  # 8

_CACHE = {}


def _paired_col(b256: int) -> int:
    """Column offset of 256-row q-block b256 in the paired SBUF layout."""
    p = min(b256, NQB - 1 - b256)
    side = 1 if b256 >= NQB // 2 else 0
    return 2 * QB * p + QB * side


def _build_nc(variant: int):
    import concourse.bacc as bacc
    import concourse.tile as tile
    import concourse.mybir as mybir
    from contextlib import ExitStack
    import collections

    F32 = mybir.dt.float32
    F32R = mybir.dt.float32r
    BF16 = mybir.dt.bfloat16
    FP8 = mybir.dt.float8e4
    EXP = mybir.ActivationFunctionType.Exp
    DR = mybir.MatmulPerfMode.DoubleRow

    s1_pairs = (0, 1, 2, 3) if variant == 0 else (4, 5, 6, 7)

    nc = bacc.Bacc(
        "TRN2",
        target_bir_lowering=False,
        debug=False,
        enable_asserts=True,
        num_devices=N_CORES // 2,
    )
    xt_d = nc.dram_tensor("xt", [C, T], BF16, kind="ExternalInput")
    wa_d = nc.dram_tensor("wa", [C, 3 * 2 * HD], BF16, kind="ExternalInput")
    wp_d = nc.dram_tensor("wp", [2 * HD, C], BF16, kind="ExternalInput")
    out_d = nc.dram_tensor("out", [T, C], BF16, kind="ExternalOutput")

    with ExitStack() as ctx:
        tc = ctx.enter_context(tile.TileContext(nc))
        singles = ctx.enter_context(tc.tile_pool(name="singles", bufs=1))
        ptpool = ctx.enter_context(tc.tile_pool(name="ptpool", bufs=3))
        rpool = ctx.enter_context(tc.tile_pool(name="rpool", bufs=4))
        opool = ctx.enter_context(tc.tile_pool(name="opool", bufs=4))
        ps = ctx.enter_context(tc.tile_pool(name="ps", bufs=2, space="PSUM"))
        ps_st = ctx.enter_context(tc.tile_pool(name="ps_st", bufs=2, space="PSUM"))
        ps_yt = ctx.enter_context(tc.tile_pool(name="ps_yt", bufs=2, space="PSUM"))

        # ---- persistent SBUF tensors ----
        xt6 = singles.tile([128, 6, T], BF16)  # x^T, c-chunk major
        qt = singles.tile([128, T], BF16)  # Q^T, paired column layout
        kt = singles.tile([128, T], BF16)  # K^T, natural column layout
        yt_all = singles.tile([128, T], BF16)  # normalized y^T, paired layout
        # V natural blocks (bf16) + ones col: [k-part, kb, slot, d+1]
        v1 = singles.tile([128, NKB, 2, HD + 1], BF16)
        wa_sb = singles.tile([128, 6, 3 * 2 * HD], BF16)
        wp_sb = singles.tile([2 * HD, C], BF16)
        ones64 = singles.tile([1, HD], F32R)
        # mbias[p, j, f] = 0 where f - p - 128j >= 0 (q >= k), else -1e30
        mbias = singles.tile([128, 2, QB], F32)

        ones_f32 = singles.tile([1, HD], F32)
        nc.gpsimd.memset(ones_f32, 1.0)
        nc.vector.tensor_copy(out=ones64, in_=ones_f32)
        vone_f32 = singles.tile([128, NKB * 2], F32)
        nc.gpsimd.memset(vone_f32, 1.0)
        nc.gpsimd.tensor_copy(
            out=v1[:, :, :, HD : HD + 1],
            in_=vone_f32.rearrange("p (a s) -> p a s", a=NKB).unsqueeze(3),
        )

        # keep -1e30 where f < p + 128j, fill 0.0 where f >= p + 128j
        nc.gpsimd.memset(mbias, -1.0e30)
        for j in range(2):
            nc.gpsimd.affine_select(
                out=mbias[:, j, :],
                in_=mbias[:, j, :],
                compare_op=mybir.AluOpType.is_gt,
                fill=0.0,
                base=128 * j,
                channel_multiplier=1,
                pattern=[[-1, QB]],
            )

        # wa as one strided DMA (HWDGE dispatch is globally serialized at
        # ~625ns/DMA, so fewer DMAs win), then the six x^T chunks spread
        # over the three DGE queues
        nc.sync.dma_start(
            out=wa_sb, in_=wa_d.ap().rearrange("(a p) f -> p a f", p=128)
        )
        xt_q = [nc.sync, nc.scalar, nc.gpsimd]
        for i in range(6):
            xt_q[i % 3].dma_start(
                out=xt6[:, i, :], in_=xt_d.ap()[i * 128 : (i + 1) * 128, :]
            )
        nc.gpsimd.dma_start(out=wp_sb, in_=wp_d.ap())

        # ---- QKV emission units (each: one PSUM group + copy-out) ----
        def emit_q(ts):
            t0 = ts * TS
            pp = ps.tile([128, TS], F32, name="pp", tag="ps")
            for ic in range(6):
                nc.tensor.matmul(
                    pp,
                    lhsT=wa_sb[:, ic, 0:128],
                    rhs=xt6[:, ic, t0 : t0 + TS],
                    start=(ic == 0),
                    stop=(ic == 5),
                )
            for half in range(2):
                col = _paired_col(2 * ts + half)
                nc.vector.tensor_copy(
                    out=qt[:, col : col + QB],
                    in_=pp[:, half * QB : (half + 1) * QB],
                )

        def emit_k(ts):
            t0 = ts * TS
            pp = ps.tile([128, TS], F32, name="pp", tag="ps")
            for ic in range(6):
                nc.tensor.matmul(
                    pp,
                    lhsT=wa_sb[:, ic, 128:256],
                    rhs=xt6[:, ic, t0 : t0 + TS],
                    start=(ic == 0),
                    stop=(ic == 5),
                )
            nc.scalar.copy(out=kt[:, t0 : t0 + TS], in_=pp)

        def emit_v(tb):
            vp = ps.tile([128, 128], F32, name="vp", tag="ps")
            for ic in range(6):
                nc.tensor.matmul(
                    vp,
                    lhsT=xt6[:, ic, tb * 128 : (tb + 1) * 128],
                    rhs=wa_sb[:, ic, 256:384],
                    start=(ic == 0),
                    stop=(ic == 5),
                )
            nc.vector.tensor_copy(
                out=v1[:, tb, :, 0:HD],
                in_=vp.rearrange("p (s d) -> p s d", s=2),
            )

        # ---- deferred-work streams ----
        scale = 1.0 / float(np.sqrt(HD))
        work_q = collections.deque()  # norm/proj closures (lag >= 1 pair)
        fillers = collections.deque()  # remaining QKV units

        # Deferred norm/proj work runs as a 2-stage pipeline: stage 1 emits
        # the PE matmuls, stage 2 (enqueued when stage 1 drains, so it pops
        # at a later group) emits their DVE consumers.  That way DVE-queue
        # entries are nearly-ready when enqueued and never head-of-line
        # block the prompt-class mask adds / staging copies.
        def drain_one():
            if fillers:
                fillers.popleft()()
            if work_q:
                work_q.popleft()()
            if len(work_q) > 6:
                work_q.popleft()()

        def emit_proj(tb, full):
            def s1():
                b256, half = tb // 2, tb % 2
                col = _paired_col(b256) + 128 * half
                r1 = 128 if full else HD
                pos = []
                for c0, c1 in ((0, 512), (512, 768)):
                    po = ps.tile([128, c1 - c0], F32, name="po", tag="ps")
                    nc.tensor.matmul(
                        po,
                        lhsT=yt_all[0:r1, col : col + 128],
                        rhs=wp_sb[0:r1, c0:c1],
                        start=True,
                        stop=True,
                    )
                    pos.append(po)

                def s2():
                    osb = opool.tile([128, C], BF16, name="osb", tag="osb")
                    for (c0, c1), po in zip(((0, 512), (512, 768)), pos):
                        nc.vector.tensor_copy(out=osb[:, c0:c1], in_=po)
                    nc.sync.dma_start(
                        out=out_d.ap()[tb * 128 : (tb + 1) * 128, :], in_=osb
                    )

                work_q.append(s2)

            return s1

        def emit_norm(ytsb, r0, r1, col, after=()):
            """after: closures enqueued once this norm's mul has been
            emitted (used to order proj emission behind yt_all writes)."""

            def s1():
                r_sb = rpool.tile([1, 2 * QB], F32R, name="r_sb", tag="r_sb", bufs=8)
                with nc.allow_low_precision(reason="softmax denom broadcast"):
                    nc.vector.reciprocal(out=r_sb, in_=ytsb[HD : HD + 1, :])
                bc = ps.tile([HD, 2 * QB], F32, name="bc", tag="ps")
                nc.tensor.matmul(bc, lhsT=ones64, rhs=r_sb, start=True, stop=True)

                def s2():
                    nc.vector.tensor_mul(
                        out=yt_all[r0:r1, col : col + 2 * QB],
                        in0=ytsb[0:HD, :],
                        in1=bc,
                    )
                    work_q.extend(after)

                work_q.append(s2)

            return s1

        def emit_pair(s, i, after=()):
            """Attention for slot s (d-rows [64s, 64s+64)), pair i."""
            r0, r1 = s * HD, (s + 1) * HD
            qcol = 2 * QB * i
            n_shared = 2 * i + 2
            n_total = NKB - 2 * i
            diag_b0 = NKB - 2 - 2 * i  # first diagonal k-block of side B
            yt = ps_yt.tile([HD + 1, 2 * QB], F32, name="yt", tag="yt")
            groups = [(g, g + 2) for g in range(0, n_shared, 2)]
            kb0 = n_shared
            while kb0 < n_total:
                n = min(4, n_total - kb0)
                groups.append((kb0, kb0 + n))
                kb0 += n

            def emit_s(grp):
                ka, kb = grp
                shared = ka < n_shared
                w = 2 * QB if shared else QB
                qoff = qcol if shared else qcol + QB
                gw = w * (kb - ka)
                st = ps_st.tile([128, 4 * QB], F32, name="st", tag="st")
                for j in range(kb - ka):
                    blk = ka + j
                    nc.tensor.matmul(
                        st[:, j * w : (j + 1) * w],
                        lhsT=kt[r0:r1, blk * KB : (blk + 1) * KB],
                        rhs=qt[r0:r1, qoff : qoff + w],
                        start=True,
                        stop=True,
                    )
                # causal mask: -1e30 bias onto the diagonal blocks' own half
                for j in range(kb - ka):
                    blk = ka + j
                    if shared and blk in (2 * i, 2 * i + 1):
                        nc.vector.tensor_add(
                            out=st[:, j * w : j * w + QB],
                            in0=st[:, j * w : j * w + QB],
                            in1=mbias[:, blk - 2 * i, :],
                        )
                    elif not shared and blk in (diag_b0, diag_b0 + 1):
                        nc.vector.tensor_add(
                            out=st[:, j * w : (j + 1) * w],
                            in0=st[:, j * w : (j + 1) * w],
                            in1=mbias[:, blk - diag_b0, :],
                        )
                pt = ptpool.tile([128, 4 * QB], BF16, name="pt", tag="pt")
                nc.scalar.activation(
                    out=pt[:, 0:gw], in_=st[:, 0:gw], func=EXP, scale=scale
                )
                return pt, w

            def emit_pv(grp, pt, w):
                ka, kb = grp
                shared = ka < n_shared
                for j in range(kb - ka):
                    blk = ka + j
                    if shared:
                        nc.tensor.matmul(
                            yt,
                            lhsT=v1[:, blk, s, :],
                            rhs=pt[:, j * w : (j + 1) * w],
                            start=(blk == 0),
                            stop=False,
                            skip_group_check=True,
                        )
                    else:
                        nc.tensor.matmul(
                            yt[:, QB : 2 * QB],
                            lhsT=v1[:, blk, s, :],
                            rhs=pt[:, j * w : (j + 1) * w],
                            start=False,
                            stop=(blk == n_total - 1),
                            skip_group_check=True,
                        )

            pending = None
            for grp in groups:
                cur = (grp, *emit_s(grp))
                if pending is not None:
                    emit_pv(*pending)
                pending = cur
                drain_one()
            emit_pv(*pending)
            # free the yt PSUM slot quickly; normalization is deferred
            ytsb = rpool.tile([HD + 1, 2 * QB], F32, name="ytsb", tag="ytsb", bufs=6)
            nc.vector.tensor_copy(out=ytsb, in_=yt)
            work_q.append(emit_norm(ytsb, r0, r1, qcol, after))

        # ---- schedule: minimal upfront QKV for pair 7; V streams
        # just-in-time; K/Q fillers carry per-pair deadlines ----
        for ts in range(5):
            emit_k(ts)
        emit_q(3)
        emit_q(4)
        for tb in range(10):
            emit_v(tb)
        # deadline -> units that must have drained before that pair starts
        # (pair i needs K ts <= (31-2i)//4 and Q ts {i//2, (15-i)//2} at
        # its start; V for kb consumed mid-pair streams via the per-group
        # drain, one deadline later)
        due = {
            6: [lambda tb=tb: emit_v(tb) for tb in range(10, 18)],
            5: [lambda: emit_k(5), lambda: emit_q(2), lambda: emit_q(5)]
            + [lambda tb=tb: emit_v(tb) for tb in range(18, 20)],
            4: [lambda tb=tb: emit_v(tb) for tb in range(20, 22)],
            3: [lambda: emit_k(6), lambda: emit_q(1), lambda: emit_q(6)]
            + [lambda tb=tb: emit_v(tb) for tb in range(22, 24)],
            2: [lambda tb=tb: emit_v(tb) for tb in range(24, 26)],
            1: [lambda: emit_k(7), lambda: emit_q(0), lambda: emit_q(7)]
            + [lambda tb=tb: emit_v(tb) for tb in range(26, 28)],
            0: [lambda tb=tb: emit_v(tb) for tb in range(28, 32)],
        }
        for i in range(6, -1, -1):
            fillers.extend(due.get(i, []))

        # max fillers allowed to remain when pair i starts = units whose
        # deadline is strictly below i (fillers are queued deadline-major)
        allowed = {
            i: sum(len(due.get(j, [])) for j in range(0, i)) for i in range(NPAIR)
        }

        for i in reversed(range(NPAIR)):
            while len(fillers) > allowed[i]:
                fillers.popleft()()
            full = i in s1_pairs
            projs = [
                emit_proj(tb, full)
                for tb in (2 * i, 2 * i + 1, NKB - 2 - 2 * i, NKB - 1 - 2 * i)
            ]
            emit_pair(0, i, after=() if full else projs)
            if full:
                emit_pair(1, i, after=projs)
        while fillers:
            fillers.popleft()()
        while work_q:
            work_q.popleft()()

    nc.compile()
    return nc


def _get_ncs():
    if "ncs" not in _CACHE:
        _CACHE["ncs"] = [_build_nc(0), _build_nc(1)]
    return _CACHE["ncs"]


def _core_inputs(x, w_attn, w_proj):
    """Build per-core input dicts (bf16, pre-transposed x, head slices)."""
    import ml_dtypes

    bf16 = ml_dtypes.bfloat16
    xt = np.ascontiguousarray(x.reshape(T, C).T.astype(bf16))
    w_attn = np.asarray(w_attn, dtype=np.float32)
    w_proj = np.asarray(w_proj, dtype=np.float32)
    in_maps = []
    for c in range(N_CORES):
        hF = c
        hH = 8 + (c % 4)
        wa = np.zeros((C, 3, 2, HD), dtype=np.float32)
        wp = np.zeros((2 * HD, C), dtype=np.float32)
        for s, h in enumerate((hF, hH)):
            for p in range(3):
                wa[:, p, s, :] = w_attn[:, p * C + h * HD : p * C + (h + 1) * HD]
            wp[s * HD : (s + 1) * HD, :] = w_proj[h * HD : (h + 1) * HD, :]
        in_maps.append(
            {
                "xt": xt,
                "wa": np.ascontiguousarray(wa.reshape(C, 3 * 2 * HD)).astype(bf16),
                "wp": wp.astype(bf16),
            }
        )
    return in_maps


def _make_sharded(nc, devices):
    """Build one 4-core shard_map'd PJRT executable for a program variant."""
    import jax
    import concourse.mybir as mybir
    from concourse import bass2jax
    from jax.experimental.shard_map import shard_map
    from jax.sharding import Mesh, PartitionSpec

    in_names, out_names, out_avals, zero_outs = [], [], [], []
    for alloc in nc.m.functions[0].allocations:
        if not isinstance(alloc, mybir.MemoryLocationSet):
            continue
        name = alloc.memorylocations[0].name
        if alloc.kind == "ExternalInput":
            if nc.partition_id_tensor and name == nc.partition_id_tensor.name:
                continue
            in_names.append(name)
        elif alloc.kind == "ExternalOutput":
            shape = tuple(alloc.tensor_shape)
            dtype = mybir.dt.np(alloc.dtype)
            out_names.append(name)
            out_avals.append(jax.core.ShapedArray(shape, dtype))
            zero_outs.append(np.zeros(shape, dtype))
    n_params = len(in_names)
    all_in_names = in_names + out_names
    if nc.partition_id_tensor:
        all_in_names = all_in_names + [nc.partition_id_tensor.name]

    def _body(*args):
        operands = list(args)
        if nc.partition_id_tensor:
            operands.append(bass2jax.partition_id_tensor())
        outs = bass2jax._bass_exec_p.bind(
            *operands,
            out_avals=tuple(out_avals),
            in_names=tuple(all_in_names),
            out_names=tuple(out_names),
            lowering_input_output_aliases=(),
            sim_require_finite=True,
            sim_require_nnan=True,
            nc=nc,
        )
        return tuple(outs)

    mesh = Mesh(np.asarray(devices), ("core",))
    n_out = len(out_names)
    donate = tuple(range(n_params, n_params + n_out))
    sharded = jax.jit(
        shard_map(
            _body,
            mesh=mesh,
            in_specs=(PartitionSpec("core"),) * (n_params + n_out),
            out_specs=(PartitionSpec("core"),) * n_out,
            check_rep=False,
        ),
        donate_argnums=donate,
        keep_unused=True,
    )
    return sharded, in_names, out_names, out_avals, zero_outs


def _get_runner():
    if "runner" in _CACHE:
        return _CACHE["runner"]
    import jax
    from concourse import bass2jax

    ncs = _get_ncs()
    bass2jax.install_neuronx_cc_hook()
    devices = jax.devices()[:N_CORES]
    execs = [
        _make_sharded(ncs[0], devices[0:4]),
        _make_sharded(ncs[1], devices[4:8]),
    ]

    def run(in_maps):
        results = [None] * N_CORES
        pending = []
        for v, (sharded, in_names, out_names, out_avals, zero_outs) in enumerate(
            execs
        ):
            cores = range(4 * v, 4 * v + 4)
            concat_in = [
                np.concatenate([in_maps[c][name] for c in cores], axis=0)
                for name in in_names
            ]
            concat_zeros = [
                np.zeros((4 * z.shape[0], *z.shape[1:]), z.dtype) for z in zero_outs
            ]
            out_arrs = sharded(*concat_in, *concat_zeros)
            pending.append((v, out_names, out_avals, out_arrs))
        for v, out_names, out_avals, out_arrs in pending:
            for i, name in enumerate(out_names):
                arr = np.asarray(out_arrs[i]).reshape(4, *out_avals[i].shape)
                for j in range(4):
                    c = 4 * v + j
                    if results[c] is None:
                        results[c] = {}
                    results[c][name] = arr[j]
        return results

    _CACHE["runner"] = run
    return run


def kernel(x, w_attn, w_proj):
    run = _get_runner()
    in_maps = _core_inputs(np.asarray(x), np.asarray(w_attn), np.asarray(w_proj))
    results = run(in_maps)
    out = np.zeros((T, C), dtype=np.float32)
    for c in range(N_CORES):
        out += results[c]["out"].astype(np.float32)
    return out.reshape(1, T, C)


# revision 46
# speedup vs baseline: 1.4172x; 1.0222x over previous
"""Causal self-attention (B=1, T=4096, C=768, H=12) on 8 Trainium2 NeuronCores.

Sharding: 24 units = (head, query-half).  Each core owns one full head
(slot 0, all 8 causal-balanced query pairs) plus one half head (slot 1,
4 pairs) -- 1.5 heads of attention area per core, no dummy slots.
Cores 0-3 run a program variant whose slot 1 covers pairs 0-3; cores
4-7 run the complementary variant (pairs 4-7).  The two variants have
identical instruction streams (only column constants differ), so their
simulated/hardware cost is identical.

Head map: core c in 0..3: slot0 = head c, slot1 = head 8+c (pairs 0-3)
          core c in 4..7: slot0 = head c, slot1 = head 4+c (pairs 4-7)

Per core:
  1. x^T arrives pre-transposed and pre-cast to bf16 by the host (a
     sharding-layout choice; no PE transposes anywhere).  Q^T/K^T are
     projected per 512-col t-slice into [128, T] (2 heads x 64 dims on
     partitions); V is projected directly in natural [t, d] layout and
     cast to fp8e4 with an appended ones column that accumulates the
     softmax denominator during P@V.  QKV work is interleaved into the
     attention stream as fillers so the Act engine starts early.
  2. Flash-style causal attention, q-blocks in balanced pairs (i, 15-i):
     S^T = K^T.T @ Q^T per 128-wide k-block (bf16).  Causal masking
     pre-writes a -1e30 bias tile into PSUM and lets the diagonal S
     matmuls accumulate onto it (start=False), keeping the mask off the
     S->exp critical chain.  P = exp(S^T/8) on the Act engine writes
     fp8 directly; P@V runs in fp8 DoubleRow mode (two k-blocks
     contracted per matmul at 0.5 cycles/row) accumulating y^T plus
     denominator in PSUM.
  3. y^T normalized via reciprocal + ones-broadcast matmul; partial
     output projection (bank-aligned 512/256 chunks through the shared
     1-bank PSUM pool); partial sums DMA'd out in bf16 and summed on
     the host.  Norm/proj run as deferred closures drained a full pair
     behind the attention front so their dependencies are always ready
     (the PE wait-queue is strict in-order; a stalled matmul blocks
     everything behind it).
"""

import sys

sys.path.insert(0, "/opt/trn_rl_repo")

import numpy as np

T = 4096
C = 768
H = 12
HD = 64
N_CORES = 8
QB = 256  # q-block rows
NQB = T // QB  # 16
KB = 128  # k-block rows
NKB = T // KB  # 32
NPAIR = NQB // 2  # 8 causal-balanced pairs (i, 15-i)
TS = 512  # t-slice for Q/K projection
NTS = T // TS  # 8

_CACHE = {}


def _paired_col(b256: int) -> int:
    """Column offset of 256-row q-block b256 in the paired SBUF layout."""
    p = min(b256, NQB - 1 - b256)
    side = 1 if b256 >= NQB // 2 else 0
    return 2 * QB * p + QB * side


def _build_nc(variant: int):
    import concourse.bacc as bacc
    import concourse.tile as tile
    import concourse.mybir as mybir
    from contextlib import ExitStack
    import collections

    F32 = mybir.dt.float32
    F32R = mybir.dt.float32r
    BF16 = mybir.dt.bfloat16
    FP8 = mybir.dt.float8e4
    EXP = mybir.ActivationFunctionType.Exp
    DR = mybir.MatmulPerfMode.DoubleRow

    s1_pairs = (0, 1, 2, 3) if variant == 0 else (4, 5, 6, 7)

    nc = bacc.Bacc(
        "TRN2",
        target_bir_lowering=False,
        debug=False,
        enable_asserts=True,
        num_devices=N_CORES // 2,
    )
    xt_d = nc.dram_tensor("xt", [C, T], BF16, kind="ExternalInput")
    wa_d = nc.dram_tensor("wa", [C, 3 * 2 * HD], BF16, kind="ExternalInput")
    wp_d = nc.dram_tensor("wp", [2 * HD, C], BF16, kind="ExternalInput")
    out_d = nc.dram_tensor("out", [T, C], BF16, kind="ExternalOutput")

    with ExitStack() as ctx:
        tc = ctx.enter_context(tile.TileContext(nc))
        singles = ctx.enter_context(tc.tile_pool(name="singles", bufs=1))
        ptpool = ctx.enter_context(tc.tile_pool(name="ptpool", bufs=3))
        rpool = ctx.enter_context(tc.tile_pool(name="rpool", bufs=4))
        opool = ctx.enter_context(tc.tile_pool(name="opool", bufs=4))
        ps = ctx.enter_context(tc.tile_pool(name="ps", bufs=2, space="PSUM"))
        ps_st = ctx.enter_context(tc.tile_pool(name="ps_st", bufs=2, space="PSUM"))
        ps_yt = ctx.enter_context(tc.tile_pool(name="ps_yt", bufs=2, space="PSUM"))

        # ---- persistent SBUF tensors ----
        xt6 = singles.tile([128, 6, T], BF16)  # x^T, c-chunk major
        qt = singles.tile([128, T], BF16)  # Q^T, paired column layout
        kt = singles.tile([128, T], BF16)  # K^T, natural column layout
        yt_all = singles.tile([128, T], BF16)  # normalized y^T, paired layout
        # V natural blocks (bf16) + ones col: [k-part, kb, slot, d+1]
        v1 = singles.tile([128, NKB, 2, HD + 1], BF16)
        wa_sb = singles.tile([128, 6, 3 * 2 * HD], BF16)
        wp_sb = singles.tile([2 * HD, C], BF16)
        ones64 = singles.tile([1, HD], F32R)
        # mbias[p, j, f] = 0 where f - p - 128j >= 0 (q >= k), else -1e30
        mbias = singles.tile([128, 2, QB], F32)

        ones_f32 = singles.tile([1, HD], F32)
        nc.gpsimd.memset(ones_f32, 1.0)
        nc.vector.tensor_copy(out=ones64, in_=ones_f32)
        vone_f32 = singles.tile([128, NKB * 2], F32)
        nc.gpsimd.memset(vone_f32, 1.0)
        nc.gpsimd.tensor_copy(
            out=v1[:, :, :, HD : HD + 1],
            in_=vone_f32.rearrange("p (a s) -> p a s", a=NKB).unsqueeze(3),
        )

        # keep -1e30 where f < p + 128j, fill 0.0 where f >= p + 128j
        nc.gpsimd.memset(mbias, -1.0e30)
        for j in range(2):
            nc.gpsimd.affine_select(
                out=mbias[:, j, :],
                in_=mbias[:, j, :],
                compare_op=mybir.AluOpType.is_gt,
                fill=0.0,
                base=128 * j,
                channel_multiplier=1,
                pattern=[[-1, QB]],
            )

        # wa as one strided DMA (HWDGE dispatch is globally serialized at
        # ~625ns/DMA, so fewer DMAs win), then the six x^T chunks spread
        # over the three DGE queues
        nc.sync.dma_start(
            out=wa_sb, in_=wa_d.ap().rearrange("(a p) f -> p a f", p=128)
        )
        # split at t=2560: the upfront K0-4/Q3-4/V0-9 groups only read
        # t < 2560, so the first exp fires ~8us earlier than waiting for
        # whole chunks (DMA transfers serialize in the shared pool)
        xt_q = [nc.sync, nc.scalar, nc.gpsimd]
        for t0, t1 in ((0, 2560), (2560, 4096)):
            for i in range(6):
                xt_q[i % 3].dma_start(
                    out=xt6[:, i, t0:t1],
                    in_=xt_d.ap()[i * 128 : (i + 1) * 128, t0:t1],
                )
        nc.gpsimd.dma_start(out=wp_sb, in_=wp_d.ap())

        # ---- QKV emission units (each: one PSUM group + copy-out) ----
        def emit_q(ts):
            t0 = ts * TS
            pp = ps.tile([128, TS], F32, name="pp", tag="ps")
            for ic in range(6):
                nc.tensor.matmul(
                    pp,
                    lhsT=wa_sb[:, ic, 0:128],
                    rhs=xt6[:, ic, t0 : t0 + TS],
                    start=(ic == 0),
                    stop=(ic == 5),
                )
            for half in range(2):
                col = _paired_col(2 * ts + half)
                nc.vector.tensor_copy(
                    out=qt[:, col : col + QB],
                    in_=pp[:, half * QB : (half + 1) * QB],
                )

        def emit_k(ts):
            t0 = ts * TS
            pp = ps.tile([128, TS], F32, name="pp", tag="ps")
            for ic in range(6):
                nc.tensor.matmul(
                    pp,
                    lhsT=wa_sb[:, ic, 128:256],
                    rhs=xt6[:, ic, t0 : t0 + TS],
                    start=(ic == 0),
                    stop=(ic == 5),
                )
            nc.vector.tensor_copy(out=kt[:, t0 : t0 + TS], in_=pp)

        def emit_v(tb):
            vp = ps.tile([128, 128], F32, name="vp", tag="ps")
            for ic in range(6):
                nc.tensor.matmul(
                    vp,
                    lhsT=xt6[:, ic, tb * 128 : (tb + 1) * 128],
                    rhs=wa_sb[:, ic, 256:384],
                    start=(ic == 0),
                    stop=(ic == 5),
                )
            nc.vector.tensor_copy(
                out=v1[:, tb, :, 0:HD],
                in_=vp.rearrange("p (s d) -> p s d", s=2),
            )

        # ---- deferred-work streams ----
        scale = 1.0 / float(np.sqrt(HD))
        work_q = collections.deque()  # norm/proj closures (lag >= 1 pair)
        fillers = collections.deque()  # remaining QKV units

        # Deferred norm/proj work runs as a 2-stage pipeline: stage 1 emits
        # the PE matmuls, stage 2 (enqueued when stage 1 drains, so it pops
        # at a later group) emits their DVE consumers.  That way DVE-queue
        # entries are nearly-ready when enqueued and never head-of-line
        # block the prompt-class mask adds / staging copies.  Nothing
        # drains in a pair's first two groups (protects the Act engine's
        # restart cadence at pair boundaries).
        def drain_one(g):
            if fillers:
                fillers.popleft()()
            if g < 2:
                return
            avail = min(2, len(work_q))  # only items queued before this group
            for _ in range(avail):
                work_q.popleft()()

        def emit_proj(tb, full):
            def s1():
                b256, half = tb // 2, tb % 2
                col = _paired_col(b256) + 128 * half
                r1 = 128 if full else HD
                pos = []
                for c0, c1 in ((0, 512), (512, 768)):
                    po = ps.tile([128, c1 - c0], F32, name="po", tag="ps")
                    nc.tensor.matmul(
                        po,
                        lhsT=yt_all[0:r1, col : col + 128],
                        rhs=wp_sb[0:r1, c0:c1],
                        start=True,
                        stop=True,
                    )
                    pos.append(po)

                def s2():
                    osb = opool.tile([128, C], BF16, name="osb", tag="osb")
                    for (c0, c1), po in zip(((0, 512), (512, 768)), pos):
                        nc.vector.tensor_copy(out=osb[:, c0:c1], in_=po)
                    nc.sync.dma_start(
                        out=out_d.ap()[tb * 128 : (tb + 1) * 128, :], in_=osb
                    )

                work_q.append(s2)

            return s1

        def emit_norm(ytsb, r0, r1, col, after=()):
            """after: closures enqueued once this norm's mul has been
            emitted (used to order proj emission behind yt_all writes)."""

            def s1():
                r_sb = rpool.tile([1, 2 * QB], F32R, name="r_sb", tag="r_sb", bufs=8)
                with nc.allow_low_precision(reason="softmax denom broadcast"):
                    nc.vector.reciprocal(out=r_sb, in_=ytsb[HD : HD + 1, :])
                bc = ps.tile([HD, 2 * QB], F32, name="bc", tag="ps")
                nc.tensor.matmul(bc, lhsT=ones64, rhs=r_sb, start=True, stop=True)

                def s2():
                    nc.vector.tensor_mul(
                        out=yt_all[r0:r1, col : col + 2 * QB],
                        in0=ytsb[0:HD, :],
                        in1=bc,
                    )
                    work_q.extend(after)

                work_q.append(s2)

            return s1

        def emit_pair(s, i, after=()):
            """Attention for slot s (d-rows [64s, 64s+64)), pair i."""
            r0, r1 = s * HD, (s + 1) * HD
            qcol = 2 * QB * i
            n_shared = 2 * i + 2
            n_total = NKB - 2 * i
            diag_b0 = NKB - 2 - 2 * i  # first diagonal k-block of side B
            yt = ps_yt.tile([HD + 1, 2 * QB], F32, name="yt", tag="yt")
            groups = [(g, g + 2) for g in range(0, n_shared, 2)]
            kb0 = n_shared
            while kb0 < n_total:
                n = min(4, n_total - kb0)
                groups.append((kb0, kb0 + n))
                kb0 += n

            def emit_s(grp):
                ka, kb = grp
                shared = ka < n_shared
                w = 2 * QB if shared else QB
                qoff = qcol if shared else qcol + QB
                gw = w * (kb - ka)
                st = ps_st.tile([128, 4 * QB], F32, name="st", tag="st")
                for j in range(kb - ka):
                    blk = ka + j
                    nc.tensor.matmul(
                        st[:, j * w : (j + 1) * w],
                        lhsT=kt[r0:r1, blk * KB : (blk + 1) * KB],
                        rhs=qt[r0:r1, qoff : qoff + w],
                        start=True,
                        stop=True,
                    )
                # causal mask: -1e30 bias onto the diagonal blocks' own half
                for j in range(kb - ka):
                    blk = ka + j
                    if shared and blk in (2 * i, 2 * i + 1):
                        nc.vector.tensor_add(
                            out=st[:, j * w : j * w + QB],
                            in0=st[:, j * w : j * w + QB],
                            in1=mbias[:, blk - 2 * i, :],
                        )
                    elif not shared and blk in (diag_b0, diag_b0 + 1):
                        nc.vector.tensor_add(
                            out=st[:, j * w : (j + 1) * w],
                            in0=st[:, j * w : (j + 1) * w],
                            in1=mbias[:, blk - diag_b0, :],
                        )
                pt = ptpool.tile([128, 4 * QB], BF16, name="pt", tag="pt")
                nc.scalar.activation(
                    out=pt[:, 0:gw], in_=st[:, 0:gw], func=EXP, scale=scale
                )
                return pt, w

            def emit_pv(grp, pt, w):
                ka, kb = grp
                shared = ka < n_shared
                for j in range(kb - ka):
                    blk = ka + j
                    if shared:
                        nc.tensor.matmul(
                            yt,
                            lhsT=v1[:, blk, s, :],
                            rhs=pt[:, j * w : (j + 1) * w],
                            start=(blk == 0),
                            stop=False,
                            skip_group_check=True,
                        )
                    else:
                        nc.tensor.matmul(
                            yt[:, QB : 2 * QB],
                            lhsT=v1[:, blk, s, :],
                            rhs=pt[:, j * w : (j + 1) * w],
                            start=False,
                            stop=(blk == n_total - 1),
                            skip_group_check=True,
                        )

            pending = None
            for gi, grp in enumerate(groups):
                cur = (grp, *emit_s(grp))
                if pending is not None:
                    emit_pv(*pending)
                pending = cur
                drain_one(gi)
            emit_pv(*pending)
            # free the yt PSUM slot quickly; normalization is deferred
            ytsb = rpool.tile([HD + 1, 2 * QB], F32, name="ytsb", tag="ytsb", bufs=6)
            nc.vector.tensor_copy(out=ytsb, in_=yt)
            work_q.append(emit_norm(ytsb, r0, r1, qcol, after))

        # ---- schedule ----
        # Pair processing order: causal-need ascending at the front (pair 7
        # needs the least K/V), and a slot0-only pair LAST so the final
        # norm->proj tail is short and runs on a warm PE.
        order = [7, 6, 5, 3, 2, 1, 0, 4]

        # upfront: exactly what pair 7 needs at its start
        for ts in range(5):
            emit_k(ts)
        emit_q(3)
        emit_q(4)
        for tb in range(10):
            emit_v(tb)

        # remaining QKV units with due POSITION in the processing order
        # (first position whose pair consumes them; V for a pair's own
        # tail k-blocks gets one position of stream-in slack)
        due = {
            1: [lambda tb=tb: emit_v(tb) for tb in range(10, 18)],
            2: [lambda: emit_q(2), lambda: emit_q(5)]
            + [lambda tb=tb: emit_v(tb) for tb in range(18, 20)],
            3: [lambda: emit_k(5), lambda: emit_k(6), lambda: emit_q(1),
                lambda: emit_q(6)]
            + [lambda tb=tb: emit_v(tb) for tb in range(20, 24)],
            4: [lambda tb=tb: emit_v(tb) for tb in range(24, 26)],
            5: [lambda: emit_k(7), lambda: emit_q(0), lambda: emit_q(7)]
            + [lambda tb=tb: emit_v(tb) for tb in range(26, 28)],
            6: [lambda tb=tb: emit_v(tb) for tb in range(28, 30)],
            7: [lambda tb=tb: emit_v(tb) for tb in range(30, 32)],
        }
        for p in range(1, NPAIR):
            fillers.extend(due.get(p, []))
        # fillers allowed to remain when position p starts = units due later
        allowed = {
            p: sum(len(due.get(j, [])) for j in range(p + 1, NPAIR))
            for p in range(NPAIR)
        }

        for p, i in enumerate(order):
            while len(fillers) > allowed[p]:
                fillers.popleft()()
            full = i in s1_pairs
            projs = [
                emit_proj(tb, full)
                for tb in (2 * i, 2 * i + 1, NKB - 2 - 2 * i, NKB - 1 - 2 * i)
            ]
            emit_pair(0, i, after=() if full else projs)
            if full:
                emit_pair(1, i, after=projs)
        while fillers:
            fillers.popleft()()
        while work_q:
            work_q.popleft()()

    nc.compile()
    return nc


def _get_ncs():
    if "ncs" not in _CACHE:
        _CACHE["ncs"] = [_build_nc(0), _build_nc(1)]
    return _CACHE["ncs"]


def _core_inputs(x, w_attn, w_proj):
    """Build per-core input dicts (bf16, pre-transposed x, head slices)."""
    import ml_dtypes

    bf16 = ml_dtypes.bfloat16
    xt = np.ascontiguousarray(x.reshape(T, C).T.astype(bf16))
    w_attn = np.asarray(w_attn, dtype=np.float32)
    w_proj = np.asarray(w_proj, dtype=np.float32)
    in_maps = []
    for c in range(N_CORES):
        hF = c
        hH = 8 + (c % 4)
        wa = np.zeros((C, 3, 2, HD), dtype=np.float32)
        wp = np.zeros((2 * HD, C), dtype=np.float32)
        for s, h in enumerate((hF, hH)):
            for p in range(3):
                wa[:, p, s, :] = w_attn[:, p * C + h * HD : p * C + (h + 1) * HD]
            wp[s * HD : (s + 1) * HD, :] = w_proj[h * HD : (h + 1) * HD, :]
        in_maps.append(
            {
                "xt": xt,
                "wa": np.ascontiguousarray(wa.reshape(C, 3 * 2 * HD)).astype(bf16),
                "wp": wp.astype(bf16),
            }
        )
    return in_maps


def _make_sharded(nc, devices):
    """Build one 4-core shard_map'd PJRT executable for a program variant."""
    import jax
    import concourse.mybir as mybir
    from concourse import bass2jax
    from jax.experimental.shard_map import shard_map
    from jax.sharding import Mesh, PartitionSpec

    in_names, out_names, out_avals, zero_outs = [], [], [], []
    for alloc in nc.m.functions[0].allocations:
        if not isinstance(alloc, mybir.MemoryLocationSet):
            continue
        name = alloc.memorylocations[0].name
        if alloc.kind == "ExternalInput":
            if nc.partition_id_tensor and name == nc.partition_id_tensor.name:
                continue
            in_names.append(name)
        elif alloc.kind == "ExternalOutput":
            shape = tuple(alloc.tensor_shape)
            dtype = mybir.dt.np(alloc.dtype)
            out_names.append(name)
            out_avals.append(jax.core.ShapedArray(shape, dtype))
            zero_outs.append(np.zeros(shape, dtype))
    n_params = len(in_names)
    all_in_names = in_names + out_names
    if nc.partition_id_tensor:
        all_in_names = all_in_names + [nc.partition_id_tensor.name]

    def _body(*args):
        operands = list(args)
        if nc.partition_id_tensor:
            operands.append(bass2jax.partition_id_tensor())
        outs = bass2jax._bass_exec_p.bind(
            *operands,
            out_avals=tuple(out_avals),
            in_names=tuple(all_in_names),
            out_names=tuple(out_names),
            lowering_input_output_aliases=(),
            sim_require_finite=True,
            sim_require_nnan=True,
            nc=nc,
        )
        return tuple(outs)

    mesh = Mesh(np.asarray(devices), ("core",))
    n_out = len(out_names)
    donate = tuple(range(n_params, n_params + n_out))
    sharded = jax.jit(
        shard_map(
            _body,
            mesh=mesh,
            in_specs=(PartitionSpec("core"),) * (n_params + n_out),
            out_specs=(PartitionSpec("core"),) * n_out,
            check_rep=False,
        ),
        donate_argnums=donate,
        keep_unused=True,
    )
    return sharded, in_names, out_names, out_avals, zero_outs


def _get_runner():
    if "runner" in _CACHE:
        return _CACHE["runner"]
    import jax
    from concourse import bass2jax

    ncs = _get_ncs()
    bass2jax.install_neuronx_cc_hook()
    devices = jax.devices()[:N_CORES]
    execs = [
        _make_sharded(ncs[0], devices[0:4]),
        _make_sharded(ncs[1], devices[4:8]),
    ]

    def run(in_maps):
        results = [None] * N_CORES
        pending = []
        for v, (sharded, in_names, out_names, out_avals, zero_outs) in enumerate(
            execs
        ):
            cores = range(4 * v, 4 * v + 4)
            concat_in = [
                np.concatenate([in_maps[c][name] for c in cores], axis=0)
                for name in in_names
            ]
            concat_zeros = [
                np.zeros((4 * z.shape[0], *z.shape[1:]), z.dtype) for z in zero_outs
            ]
            out_arrs = sharded(*concat_in, *concat_zeros)
            pending.append((v, out_names, out_avals, out_arrs))
        for v, out_names, out_avals, out_arrs in pending:
            for i, name in enumerate(out_names):
                arr = np.asarray(out_arrs[i]).reshape(4, *out_avals[i].shape)
                for j in range(4):
                    c = 4 * v + j
                    if results[c] is None:
                        results[c] = {}
                    results[c][name] = arr[j]
        return results

    _CACHE["runner"] = run
    return run


def kernel(x, w_attn, w_proj):
    run = _get_runner()
    in_maps = _core_inputs(np.asarray(x), np.asarray(w_attn), np.asarray(w_proj))
    results = run(in_maps)
    out = np.zeros((T, C), dtype=np.float32)
    for c in range(N_CORES):
        out += results[c]["out"].astype(np.float32)
    return out.reshape(1, T, C)


# revision 51
# speedup vs baseline: 1.4255x; 1.0059x over previous
"""Causal self-attention (B=1, T=4096, C=768, H=12) on 8 Trainium2 NeuronCores.

Sharding: 24 units = (head, query-half).  Each core owns one full head
(slot 0, all 8 causal-balanced query pairs) plus one half head (slot 1,
4 pairs) -- 1.5 heads of attention area per core, no dummy slots.
Cores 0-3 run a program variant whose slot 1 covers pairs 0-3; cores
4-7 run the complementary variant (pairs 4-7).  The two variants have
identical instruction streams (only column constants differ), so their
simulated/hardware cost is identical.

Head map: core c in 0..3: slot0 = head c, slot1 = head 8+c (pairs 0-3)
          core c in 4..7: slot0 = head c, slot1 = head 4+c (pairs 4-7)

Per core:
  1. x^T arrives pre-transposed and pre-cast to bf16 by the host (a
     sharding-layout choice; no PE transposes anywhere).  Q^T/K^T are
     projected per 512-col t-slice into [128, T] (2 heads x 64 dims on
     partitions); V is projected directly in natural [t, d] layout with
     an appended ones column that accumulates the softmax denominator
     during P@V.  QKV work beyond what the first pair needs streams
     into the attention phase as fillers (the serialized x^T DMA makes
     the start window precious).
  2. Flash-style causal attention, q-blocks in balanced pairs (i, 15-i):
     S^T = K^T.T @ Q^T per 128-wide k-block (bf16), causal masking via
     a -1e30 bias add on the diagonal blocks (DVE), P = exp(S^T/8) on
     the Act engine (the kernel's bottleneck: ~107us of exp at 1
     col/cycle), P@V per k-block in bf16 accumulating y^T + denominator
     in PSUM.  (fp8 DoubleRow P@V was tried and reverted: DoubleRow
     ldweights cap the output at 64 partitions, evicting the
     denominator row, and a separate denominator matmul costs the
     savings back.  GPSIMD cannot touch PSUM, so staging rides DVE.)
  3. y^T normalized via reciprocal + ones-broadcast matmul; partial
     output projection (bank-aligned 512/256 chunks through the shared
     1-bank PSUM pool); partial sums DMA'd out in bf16 and summed on
     the host.  Norm/proj run as deferred closures drained a full pair
     behind the attention front so their dependencies are always ready
     (the PE wait-queue is strict in-order; a stalled matmul blocks
     everything behind it).
"""

import sys

sys.path.insert(0, "/opt/trn_rl_repo")

import numpy as np

T = 4096
C = 768
H = 12
HD = 64
N_CORES = 8
QB = 256  # q-block rows
NQB = T // QB  # 16
KB = 128  # k-block rows
NKB = T // KB  # 32
NPAIR = NQB // 2  # 8 causal-balanced pairs (i, 15-i)
TS = 512  # t-slice for Q/K projection
NTS = T // TS  # 8

_CACHE = {}


def _paired_col(b256: int) -> int:
    """Column offset of 256-row q-block b256 in the paired SBUF layout."""
    p = min(b256, NQB - 1 - b256)
    side = 1 if b256 >= NQB // 2 else 0
    return 2 * QB * p + QB * side


def _build_nc(variant: int):
    import concourse.bacc as bacc
    import concourse.tile as tile
    import concourse.mybir as mybir
    from contextlib import ExitStack
    import collections

    F32 = mybir.dt.float32
    F32R = mybir.dt.float32r
    BF16 = mybir.dt.bfloat16
    FP8 = mybir.dt.float8e4
    EXP = mybir.ActivationFunctionType.Exp
    DR = mybir.MatmulPerfMode.DoubleRow

    s1_pairs = (0, 1, 2, 3) if variant == 0 else (4, 5, 6, 7)

    nc = bacc.Bacc(
        "TRN2",
        target_bir_lowering=False,
        debug=False,
        enable_asserts=True,
        num_devices=N_CORES // 2,
    )
    xt_d = nc.dram_tensor("xt", [C, T], BF16, kind="ExternalInput")
    wa_d = nc.dram_tensor("wa", [C, 3 * 2 * HD], BF16, kind="ExternalInput")
    wp_d = nc.dram_tensor("wp", [2 * HD, C], BF16, kind="ExternalInput")
    out_d = nc.dram_tensor("out", [T, C], BF16, kind="ExternalOutput")

    with ExitStack() as ctx:
        tc = ctx.enter_context(tile.TileContext(nc))
        singles = ctx.enter_context(tc.tile_pool(name="singles", bufs=1))
        ptpool = ctx.enter_context(tc.tile_pool(name="ptpool", bufs=3))
        rpool = ctx.enter_context(tc.tile_pool(name="rpool", bufs=4))
        opool = ctx.enter_context(tc.tile_pool(name="opool", bufs=4))
        ps = ctx.enter_context(tc.tile_pool(name="ps", bufs=2, space="PSUM"))
        ps_st = ctx.enter_context(tc.tile_pool(name="ps_st", bufs=2, space="PSUM"))
        ps_yt = ctx.enter_context(tc.tile_pool(name="ps_yt", bufs=2, space="PSUM"))

        # ---- persistent SBUF tensors ----
        xt6 = singles.tile([128, 6, T], BF16)  # x^T, c-chunk major
        qt = singles.tile([128, T], BF16)  # Q^T, paired column layout
        kt = singles.tile([128, T], BF16)  # K^T, natural column layout
        yt_all = singles.tile([128, T], BF16)  # normalized y^T, paired layout
        # V natural blocks (bf16) + ones col: [k-part, kb, slot, d+1]
        v1 = singles.tile([128, NKB, 2, HD + 1], BF16)
        wa_sb = singles.tile([128, 6, 3 * 2 * HD], BF16)
        wp_sb = singles.tile([2 * HD, C], BF16)
        ones64 = singles.tile([1, HD], F32R)
        # mbias[p, j, f] = 0 where f - p - 128j >= 0 (q >= k), else -1e30
        mbias = singles.tile([128, 2, QB], F32)

        ones_f32 = singles.tile([1, HD], F32)
        nc.gpsimd.memset(ones_f32, 1.0)
        nc.vector.tensor_copy(out=ones64, in_=ones_f32)
        vone_f32 = singles.tile([128, NKB * 2], F32)
        nc.gpsimd.memset(vone_f32, 1.0)
        nc.gpsimd.tensor_copy(
            out=v1[:, :, :, HD : HD + 1],
            in_=vone_f32.rearrange("p (a s) -> p a s", a=NKB).unsqueeze(3),
        )

        # keep -1e30 where f < p + 128j, fill 0.0 where f >= p + 128j
        nc.gpsimd.memset(mbias, -1.0e30)
        for j in range(2):
            nc.gpsimd.affine_select(
                out=mbias[:, j, :],
                in_=mbias[:, j, :],
                compare_op=mybir.AluOpType.is_gt,
                fill=0.0,
                base=128 * j,
                channel_multiplier=1,
                pattern=[[-1, QB]],
            )

        # wa as one strided DMA (HWDGE dispatch is globally serialized at
        # ~625ns/DMA, so fewer DMAs win), then the six x^T chunks spread
        # over the three DGE queues
        nc.sync.dma_start(
            out=wa_sb, in_=wa_d.ap().rearrange("(a p) f -> p a f", p=128)
        )
        # split at t=2560: the upfront K0-4/Q3-4/V0-9 groups only read
        # t < 2560, so the first exp fires ~8us earlier than waiting for
        # whole chunks (DMA transfers serialize in the shared pool)
        xt_q = [nc.sync, nc.scalar, nc.gpsimd]
        for t0, t1 in ((0, 2560), (2560, 4096)):
            for i in range(6):
                xt_q[i % 3].dma_start(
                    out=xt6[:, i, t0:t1],
                    in_=xt_d.ap()[i * 128 : (i + 1) * 128, t0:t1],
                )
        nc.gpsimd.dma_start(out=wp_sb, in_=wp_d.ap())

        # ---- QKV emission units (each: one PSUM group + copy-out) ----
        def emit_q(ts):
            t0 = ts * TS
            pp = ps.tile([128, TS], F32, name="pp", tag="ps")
            for ic in range(6):
                nc.tensor.matmul(
                    pp,
                    lhsT=wa_sb[:, ic, 0:128],
                    rhs=xt6[:, ic, t0 : t0 + TS],
                    start=(ic == 0),
                    stop=(ic == 5),
                )
            for half in range(2):
                col = _paired_col(2 * ts + half)
                nc.vector.tensor_copy(
                    out=qt[:, col : col + QB],
                    in_=pp[:, half * QB : (half + 1) * QB],
                )

        def emit_k(ts):
            t0 = ts * TS
            pp = ps.tile([128, TS], F32, name="pp", tag="ps")
            for ic in range(6):
                nc.tensor.matmul(
                    pp,
                    lhsT=wa_sb[:, ic, 128:256],
                    rhs=xt6[:, ic, t0 : t0 + TS],
                    start=(ic == 0),
                    stop=(ic == 5),
                )
            nc.vector.tensor_copy(out=kt[:, t0 : t0 + TS], in_=pp)

        def emit_v(tb):
            vp = ps.tile([128, 128], F32, name="vp", tag="ps")
            for ic in range(6):
                nc.tensor.matmul(
                    vp,
                    lhsT=xt6[:, ic, tb * 128 : (tb + 1) * 128],
                    rhs=wa_sb[:, ic, 256:384],
                    start=(ic == 0),
                    stop=(ic == 5),
                )
            nc.vector.tensor_copy(
                out=v1[:, tb, :, 0:HD],
                in_=vp.rearrange("p (s d) -> p s d", s=2),
            )

        # ---- deferred-work streams ----
        scale = 1.0 / float(np.sqrt(HD))
        work_q = collections.deque()  # norm/proj closures (lag >= 1 pair)
        fillers = collections.deque()  # remaining QKV units

        # Deferred norm/proj work runs as a 2-stage pipeline: stage 1 emits
        # the PE matmuls, stage 2 (enqueued when stage 1 drains, so it pops
        # at a later group) emits their DVE consumers.  That way DVE-queue
        # entries are nearly-ready when enqueued and never head-of-line
        # block the prompt-class mask adds / staging copies.  Nothing
        # drains in a pair's first two groups (protects the Act engine's
        # restart cadence at pair boundaries).
        def drain_one(g):
            if fillers:
                fillers.popleft()()
            if g < 2:
                return
            # throttle injections: one deferred item per group keeps the
            # exp cadence close to back-to-back; a second only under queue
            # pressure (the remainder drains in the run-out)
            n = 2 if len(work_q) > 8 else min(1, len(work_q))
            for _ in range(min(n, len(work_q))):
                work_q.popleft()()

        def emit_proj(tb, full):
            def s1():
                b256, half = tb // 2, tb % 2
                col = _paired_col(b256) + 128 * half
                r1 = 128 if full else HD
                pos = []
                for c0, c1 in ((0, 512), (512, 768)):
                    po = ps.tile([128, c1 - c0], F32, name="po", tag="ps")
                    nc.tensor.matmul(
                        po,
                        lhsT=yt_all[0:r1, col : col + 128],
                        rhs=wp_sb[0:r1, c0:c1],
                        start=True,
                        stop=True,
                    )
                    pos.append(po)

                def s2():
                    osb = opool.tile([128, C], BF16, name="osb", tag="osb")
                    for (c0, c1), po in zip(((0, 512), (512, 768)), pos):
                        nc.vector.tensor_copy(out=osb[:, c0:c1], in_=po)
                    nc.sync.dma_start(
                        out=out_d.ap()[tb * 128 : (tb + 1) * 128, :], in_=osb
                    )

                work_q.append(s2)

            return s1

        def emit_norm(ytsb, r0, r1, col, after=()):
            """after: closures enqueued once this norm's mul has been
            emitted (used to order proj emission behind yt_all writes)."""

            def s1():
                r_sb = rpool.tile([1, 2 * QB], F32R, name="r_sb", tag="r_sb", bufs=8)
                with nc.allow_low_precision(reason="softmax denom broadcast"):
                    nc.vector.reciprocal(out=r_sb, in_=ytsb[HD : HD + 1, :])
                bc = ps.tile([HD, 2 * QB], F32, name="bc", tag="ps")
                nc.tensor.matmul(bc, lhsT=ones64, rhs=r_sb, start=True, stop=True)

                def s2():
                    nc.vector.tensor_mul(
                        out=yt_all[r0:r1, col : col + 2 * QB],
                        in0=ytsb[0:HD, :],
                        in1=bc,
                    )
                    work_q.extend(after)

                work_q.append(s2)

            return s1

        def emit_pair(s, i, after=()):
            """Attention for slot s (d-rows [64s, 64s+64)), pair i."""
            r0, r1 = s * HD, (s + 1) * HD
            qcol = 2 * QB * i
            n_shared = 2 * i + 2
            n_total = NKB - 2 * i
            diag_b0 = NKB - 2 - 2 * i  # first diagonal k-block of side B
            yt = ps_yt.tile([HD + 1, 2 * QB], F32, name="yt", tag="yt")
            groups = [(g, g + 2) for g in range(0, n_shared, 2)]
            kb0 = n_shared
            while kb0 < n_total:
                n = min(4, n_total - kb0)
                groups.append((kb0, kb0 + n))
                kb0 += n

            def emit_s(grp):
                ka, kb = grp
                shared = ka < n_shared
                w = 2 * QB if shared else QB
                qoff = qcol if shared else qcol + QB
                gw = w * (kb - ka)
                st = ps_st.tile([128, 4 * QB], F32, name="st", tag="st")
                for j in range(kb - ka):
                    blk = ka + j
                    nc.tensor.matmul(
                        st[:, j * w : (j + 1) * w],
                        lhsT=kt[r0:r1, blk * KB : (blk + 1) * KB],
                        rhs=qt[r0:r1, qoff : qoff + w],
                        start=True,
                        stop=True,
                    )
                # causal mask: -1e30 bias onto the diagonal blocks' own half
                for j in range(kb - ka):
                    blk = ka + j
                    if shared and blk in (2 * i, 2 * i + 1):
                        nc.vector.tensor_add(
                            out=st[:, j * w : j * w + QB],
                            in0=st[:, j * w : j * w + QB],
                            in1=mbias[:, blk - 2 * i, :],
                        )
                    elif not shared and blk in (diag_b0, diag_b0 + 1):
                        nc.vector.tensor_add(
                            out=st[:, j * w : (j + 1) * w],
                            in0=st[:, j * w : (j + 1) * w],
                            in1=mbias[:, blk - diag_b0, :],
                        )
                pt = ptpool.tile([128, 4 * QB], BF16, name="pt", tag="pt")
                nc.scalar.activation(
                    out=pt[:, 0:gw], in_=st[:, 0:gw], func=EXP, scale=scale
                )
                return pt, w

            def emit_pv(grp, pt, w):
                ka, kb = grp
                shared = ka < n_shared
                for j in range(kb - ka):
                    blk = ka + j
                    if shared:
                        nc.tensor.matmul(
                            yt,
                            lhsT=v1[:, blk, s, :],
                            rhs=pt[:, j * w : (j + 1) * w],
                            start=(blk == 0),
                            stop=False,
                            skip_group_check=True,
                        )
                    else:
                        nc.tensor.matmul(
                            yt[:, QB : 2 * QB],
                            lhsT=v1[:, blk, s, :],
                            rhs=pt[:, j * w : (j + 1) * w],
                            start=False,
                            stop=(blk == n_total - 1),
                            skip_group_check=True,
                        )

            pending = None
            for gi, grp in enumerate(groups):
                cur = (grp, *emit_s(grp))
                if pending is not None:
                    emit_pv(*pending)
                pending = cur
                drain_one(gi)
            emit_pv(*pending)
            # free the yt PSUM slot quickly; normalization is deferred
            ytsb = rpool.tile([HD + 1, 2 * QB], F32, name="ytsb", tag="ytsb", bufs=6)
            nc.vector.tensor_copy(out=ytsb, in_=yt)
            work_q.append(emit_norm(ytsb, r0, r1, qcol, after))

        # ---- schedule ----
        # Pair processing order: causal-need ascending at the front (pair 7
        # needs the least K/V), and a slot0-only pair LAST so the final
        # norm->proj tail is short and runs on a warm PE.
        order = [7, 6, 5, 3, 2, 1, 0, 4]

        # upfront: exactly what pair 7 needs at its start
        for ts in range(5):
            emit_k(ts)
        emit_q(3)
        emit_q(4)
        for tb in range(10):
            emit_v(tb)

        # remaining QKV units with due POSITION in the processing order
        # (first position whose pair consumes them; V for a pair's own
        # tail k-blocks gets one position of stream-in slack)
        due = {
            1: [lambda tb=tb: emit_v(tb) for tb in range(10, 18)],
            2: [lambda: emit_q(2), lambda: emit_q(5)]
            + [lambda tb=tb: emit_v(tb) for tb in range(18, 20)],
            3: [lambda: emit_k(5), lambda: emit_k(6), lambda: emit_q(1),
                lambda: emit_q(6)]
            + [lambda tb=tb: emit_v(tb) for tb in range(20, 24)],
            4: [lambda tb=tb: emit_v(tb) for tb in range(24, 26)],
            5: [lambda: emit_k(7), lambda: emit_q(0), lambda: emit_q(7)]
            + [lambda tb=tb: emit_v(tb) for tb in range(26, 28)],
            6: [lambda tb=tb: emit_v(tb) for tb in range(28, 30)],
            7: [lambda tb=tb: emit_v(tb) for tb in range(30, 32)],
        }
        for p in range(1, NPAIR):
            fillers.extend(due.get(p, []))
        # fillers allowed to remain when position p starts = units due later
        allowed = {
            p: sum(len(due.get(j, [])) for j in range(p + 1, NPAIR))
            for p in range(NPAIR)
        }

        for p, i in enumerate(order):
            while len(fillers) > allowed[p]:
                fillers.popleft()()
            full = i in s1_pairs
            projs = [
                emit_proj(tb, full)
                for tb in (2 * i, 2 * i + 1, NKB - 2 - 2 * i, NKB - 1 - 2 * i)
            ]
            emit_pair(0, i, after=() if full else projs)
            if full:
                emit_pair(1, i, after=projs)
        while fillers:
            fillers.popleft()()
        while work_q:
            work_q.popleft()()

    nc.compile()
    return nc


def _get_ncs():
    if "ncs" not in _CACHE:
        _CACHE["ncs"] = [_build_nc(0), _build_nc(1)]
    return _CACHE["ncs"]


def _core_inputs(x, w_attn, w_proj):
    """Build per-core input dicts (bf16, pre-transposed x, head slices)."""
    import ml_dtypes

    bf16 = ml_dtypes.bfloat16
    xt = np.ascontiguousarray(x.reshape(T, C).T.astype(bf16))
    w_attn = np.asarray(w_attn, dtype=np.float32)
    w_proj = np.asarray(w_proj, dtype=np.float32)
    in_maps = []
    for c in range(N_CORES):
        hF = c
        hH = 8 + (c % 4)
        wa = np.zeros((C, 3, 2, HD), dtype=np.float32)
        wp = np.zeros((2 * HD, C), dtype=np.float32)
        for s, h in enumerate((hF, hH)):
            for p in range(3):
                wa[:, p, s, :] = w_attn[:, p * C + h * HD : p * C + (h + 1) * HD]
            wp[s * HD : (s + 1) * HD, :] = w_proj[h * HD : (h + 1) * HD, :]
        in_maps.append(
            {
                "xt": xt,
                "wa": np.ascontiguousarray(wa.reshape(C, 3 * 2 * HD)).astype(bf16),
                "wp": wp.astype(bf16),
            }
        )
    return in_maps


def _make_sharded(nc, devices):
    """Build one 4-core shard_map'd PJRT executable for a program variant."""
    import jax
    import concourse.mybir as mybir
    from concourse import bass2jax
    from jax.experimental.shard_map import shard_map
    from jax.sharding import Mesh, PartitionSpec

    in_names, out_names, out_avals, zero_outs = [], [], [], []
    for alloc in nc.m.functions[0].allocations:
        if not isinstance(alloc, mybir.MemoryLocationSet):
            continue
        name = alloc.memorylocations[0].name
        if alloc.kind == "ExternalInput":
            if nc.partition_id_tensor and name == nc.partition_id_tensor.name:
                continue
            in_names.append(name)
        elif alloc.kind == "ExternalOutput":
            shape = tuple(alloc.tensor_shape)
            dtype = mybir.dt.np(alloc.dtype)
            out_names.append(name)
            out_avals.append(jax.core.ShapedArray(shape, dtype))
            zero_outs.append(np.zeros(shape, dtype))
    n_params = len(in_names)
    all_in_names = in_names + out_names
    if nc.partition_id_tensor:
        all_in_names = all_in_names + [nc.partition_id_tensor.name]

    def _body(*args):
        operands = list(args)
        if nc.partition_id_tensor:
            operands.append(bass2jax.partition_id_tensor())
        outs = bass2jax._bass_exec_p.bind(
            *operands,
            out_avals=tuple(out_avals),
            in_names=tuple(all_in_names),
            out_names=tuple(out_names),
            lowering_input_output_aliases=(),
            sim_require_finite=True,
            sim_require_nnan=True,
            nc=nc,
        )
        return tuple(outs)

    mesh = Mesh(np.asarray(devices), ("core",))
    n_out = len(out_names)
    donate = tuple(range(n_params, n_params + n_out))
    sharded = jax.jit(
        shard_map(
            _body,
            mesh=mesh,
            in_specs=(PartitionSpec("core"),) * (n_params + n_out),
            out_specs=(PartitionSpec("core"),) * n_out,
            check_rep=False,
        ),
        donate_argnums=donate,
        keep_unused=True,
    )
    return sharded, in_names, out_names, out_avals, zero_outs


def _get_runner():
    if "runner" in _CACHE:
        return _CACHE["runner"]
    import jax
    from concourse import bass2jax

    ncs = _get_ncs()
    bass2jax.install_neuronx_cc_hook()
    devices = jax.devices()[:N_CORES]
    execs = [
        _make_sharded(ncs[0], devices[0:4]),
        _make_sharded(ncs[1], devices[4:8]),
    ]

    def run(in_maps):
        results = [None] * N_CORES
        pending = []
        for v, (sharded, in_names, out_names, out_avals, zero_outs) in enumerate(
            execs
        ):
            cores = range(4 * v, 4 * v + 4)
            concat_in = [
                np.concatenate([in_maps[c][name] for c in cores], axis=0)
                for name in in_names
            ]
            concat_zeros = [
                np.zeros((4 * z.shape[0], *z.shape[1:]), z.dtype) for z in zero_outs
            ]
            out_arrs = sharded(*concat_in, *concat_zeros)
            pending.append((v, out_names, out_avals, out_arrs))
        for v, out_names, out_avals, out_arrs in pending:
            for i, name in enumerate(out_names):
                arr = np.asarray(out_arrs[i]).reshape(4, *out_avals[i].shape)
                for j in range(4):
                    c = 4 * v + j
                    if results[c] is None:
                        results[c] = {}
                    results[c][name] = arr[j]
        return results

    _CACHE["runner"] = run
    return run


def kernel(x, w_attn, w_proj):
    run = _get_runner()
    in_maps = _core_inputs(np.asarray(x), np.asarray(w_attn), np.asarray(w_proj))
    results = run(in_maps)
    out = np.zeros((T, C), dtype=np.float32)
    for c in range(N_CORES):
        out += results[c]["out"].astype(np.float32)
    return out.reshape(1, T, C)


# revision 52
# speedup vs baseline: 1.4489x; 1.0164x over previous
"""Causal self-attention (B=1, T=4096, C=768, H=12) on 8 Trainium2 NeuronCores.

Sharding: 24 units = (head, query-half).  Each core owns one full head
(slot 0, all 8 causal-balanced query pairs) plus one half head (slot 1,
4 pairs) -- 1.5 heads of attention area per core, no dummy slots.
Cores 0-3 run a program variant whose slot 1 covers pairs 0-3; cores
4-7 run the complementary variant (pairs 4-7).  The two variants have
identical instruction streams (only column constants differ), so their
simulated/hardware cost is identical.

Head map: core c in 0..3: slot0 = head c, slot1 = head 8+c (pairs 0-3)
          core c in 4..7: slot0 = head c, slot1 = head 4+c (pairs 4-7)

Per core:
  1. x^T arrives pre-transposed and pre-cast to bf16 by the host (a
     sharding-layout choice; no PE transposes anywhere).  Q^T/K^T are
     projected per 512-col t-slice into [128, T] (2 heads x 64 dims on
     partitions); V is projected directly in natural [t, d] layout with
     an appended ones column that accumulates the softmax denominator
     during P@V.  QKV work beyond what the first pair needs streams
     into the attention phase as fillers (the serialized x^T DMA makes
     the start window precious).
  2. Flash-style causal attention, q-blocks in balanced pairs (i, 15-i):
     S^T = K^T.T @ Q^T per 128-wide k-block (bf16), causal masking via
     a -1e30 bias add on the diagonal blocks (DVE), P = exp(S^T/8) on
     the Act engine (the kernel's bottleneck: ~107us of exp at 1
     col/cycle), P@V per k-block in bf16 accumulating y^T + denominator
     in PSUM.  (fp8 DoubleRow P@V was tried and reverted: DoubleRow
     ldweights cap the output at 64 partitions, evicting the
     denominator row, and a separate denominator matmul costs the
     savings back.  GPSIMD cannot touch PSUM, so staging rides DVE.)
  3. y^T normalized via reciprocal + ones-broadcast matmul; partial
     output projection (bank-aligned 512/256 chunks through the shared
     1-bank PSUM pool); partial sums DMA'd out in bf16 and summed on
     the host.  Norm/proj run as deferred closures drained a full pair
     behind the attention front so their dependencies are always ready
     (the PE wait-queue is strict in-order; a stalled matmul blocks
     everything behind it).
"""

import sys

sys.path.insert(0, "/opt/trn_rl_repo")

import numpy as np

T = 4096
C = 768
H = 12
HD = 64
N_CORES = 8
QB = 256  # q-block rows
NQB = T // QB  # 16
KB = 128  # k-block rows
NKB = T // KB  # 32
NPAIR = NQB // 2  # 8 causal-balanced pairs (i, 15-i)
TS = 512  # t-slice for Q/K projection
NTS = T // TS  # 8

_CACHE = {}


def _paired_col(b256: int) -> int:
    """Column offset of 256-row q-block b256 in the paired SBUF layout."""
    p = min(b256, NQB - 1 - b256)
    side = 1 if b256 >= NQB // 2 else 0
    return 2 * QB * p + QB * side


def _build_nc(variant: int):
    import concourse.bacc as bacc
    import concourse.tile as tile
    import concourse.mybir as mybir
    from contextlib import ExitStack
    import collections

    F32 = mybir.dt.float32
    F32R = mybir.dt.float32r
    BF16 = mybir.dt.bfloat16
    FP8 = mybir.dt.float8e4
    EXP = mybir.ActivationFunctionType.Exp
    DR = mybir.MatmulPerfMode.DoubleRow

    s1_pairs = (0, 1, 2, 3) if variant == 0 else (4, 5, 6, 7)

    nc = bacc.Bacc(
        "TRN2",
        target_bir_lowering=False,
        debug=False,
        enable_asserts=True,
        num_devices=N_CORES // 2,
    )
    xt_d = nc.dram_tensor("xt", [C, T], BF16, kind="ExternalInput")
    wa_d = nc.dram_tensor("wa", [C, 3 * 2 * HD], BF16, kind="ExternalInput")
    wp_d = nc.dram_tensor("wp", [2 * HD, C], BF16, kind="ExternalInput")
    out_d = nc.dram_tensor("out", [T, C], BF16, kind="ExternalOutput")

    with ExitStack() as ctx:
        tc = ctx.enter_context(tile.TileContext(nc))
        singles = ctx.enter_context(tc.tile_pool(name="singles", bufs=1))
        ptpool = ctx.enter_context(tc.tile_pool(name="ptpool", bufs=3))
        rpool = ctx.enter_context(tc.tile_pool(name="rpool", bufs=4))
        opool = ctx.enter_context(tc.tile_pool(name="opool", bufs=4))
        ps = ctx.enter_context(tc.tile_pool(name="ps", bufs=2, space="PSUM"))
        ps_st = ctx.enter_context(tc.tile_pool(name="ps_st", bufs=2, space="PSUM"))
        ps_yt = ctx.enter_context(tc.tile_pool(name="ps_yt", bufs=2, space="PSUM"))

        # ---- persistent SBUF tensors ----
        xt6 = singles.tile([128, 6, T], BF16)  # x^T, c-chunk major
        qt = singles.tile([128, T], BF16)  # Q^T, paired column layout
        kt = singles.tile([128, T], BF16)  # K^T, natural column layout
        yt_all = singles.tile([128, T], BF16)  # normalized y^T, paired layout
        # V natural blocks (bf16) + ones col: [k-part, kb, slot, d+1]
        v1 = singles.tile([128, NKB, 2, HD + 1], BF16)
        wa_sb = singles.tile([128, 6, 3 * 2 * HD], BF16)
        wp_sb = singles.tile([2 * HD, C], BF16)
        ones64 = singles.tile([1, HD], F32R)
        # mbias[p, j, f] = 0 where f - p - 128j >= 0 (q >= k), else -1e30
        mbias = singles.tile([128, 2, QB], F32)

        ones_f32 = singles.tile([1, HD], F32)
        nc.gpsimd.memset(ones_f32, 1.0)
        nc.vector.tensor_copy(out=ones64, in_=ones_f32)
        vone_f32 = singles.tile([128, NKB * 2], F32)
        nc.gpsimd.memset(vone_f32, 1.0)
        nc.gpsimd.tensor_copy(
            out=v1[:, :, :, HD : HD + 1],
            in_=vone_f32.rearrange("p (a s) -> p a s", a=NKB).unsqueeze(3),
        )

        # keep -1e30 where f < p + 128j, fill 0.0 where f >= p + 128j
        nc.gpsimd.memset(mbias, -1.0e30)
        for j in range(2):
            nc.gpsimd.affine_select(
                out=mbias[:, j, :],
                in_=mbias[:, j, :],
                compare_op=mybir.AluOpType.is_gt,
                fill=0.0,
                base=128 * j,
                channel_multiplier=1,
                pattern=[[-1, QB]],
            )

        # wa as one strided DMA (HWDGE dispatch is globally serialized at
        # ~625ns/DMA, so fewer DMAs win), then the six x^T chunks spread
        # over the three DGE queues
        nc.sync.dma_start(
            out=wa_sb, in_=wa_d.ap().rearrange("(a p) f -> p a f", p=128)
        )
        # split at t=2560: the upfront K0-4/Q3-4/V0-9 groups only read
        # t < 2560, so the first exp fires ~8us earlier than waiting for
        # whole chunks (DMA transfers serialize in the shared pool)
        xt_q = [nc.sync, nc.scalar, nc.gpsimd]
        for t0, t1 in ((0, 2560), (2560, 4096)):
            for i in range(6):
                xt_q[i % 3].dma_start(
                    out=xt6[:, i, t0:t1],
                    in_=xt_d.ap()[i * 128 : (i + 1) * 128, t0:t1],
                )
        nc.gpsimd.dma_start(out=wp_sb, in_=wp_d.ap())

        # ---- QKV emission units (each: one PSUM group + copy-out) ----
        def emit_q(ts):
            t0 = ts * TS
            pp = ps.tile([128, TS], F32, name="pp", tag="ps")
            for ic in range(6):
                nc.tensor.matmul(
                    pp,
                    lhsT=wa_sb[:, ic, 0:128],
                    rhs=xt6[:, ic, t0 : t0 + TS],
                    start=(ic == 0),
                    stop=(ic == 5),
                )
            for half in range(2):
                col = _paired_col(2 * ts + half)
                nc.vector.tensor_copy(
                    out=qt[:, col : col + QB],
                    in_=pp[:, half * QB : (half + 1) * QB],
                )

        def emit_k(ts):
            t0 = ts * TS
            pp = ps.tile([128, TS], F32, name="pp", tag="ps")
            for ic in range(6):
                nc.tensor.matmul(
                    pp,
                    lhsT=wa_sb[:, ic, 128:256],
                    rhs=xt6[:, ic, t0 : t0 + TS],
                    start=(ic == 0),
                    stop=(ic == 5),
                )
            nc.vector.tensor_copy(out=kt[:, t0 : t0 + TS], in_=pp)

        def emit_v(tb):
            vp = ps.tile([128, 128], F32, name="vp", tag="ps")
            for ic in range(6):
                nc.tensor.matmul(
                    vp,
                    lhsT=xt6[:, ic, tb * 128 : (tb + 1) * 128],
                    rhs=wa_sb[:, ic, 256:384],
                    start=(ic == 0),
                    stop=(ic == 5),
                )
            nc.vector.tensor_copy(
                out=v1[:, tb, :, 0:HD],
                in_=vp.rearrange("p (s d) -> p s d", s=2),
            )

        # ---- deferred-work streams ----
        scale = 1.0 / float(np.sqrt(HD))
        work_q = collections.deque()  # norm/proj closures (lag >= 1 pair)
        fillers = collections.deque()  # remaining QKV units

        # Deferred norm/proj work runs as a 2-stage pipeline: stage 1 emits
        # the PE matmuls, stage 2 (enqueued when stage 1 drains, so it pops
        # at a later group) emits their DVE consumers.  That way DVE-queue
        # entries are nearly-ready when enqueued and never head-of-line
        # block the prompt-class mask adds / staging copies.  Nothing
        # drains in a pair's first two groups (protects the Act engine's
        # restart cadence at pair boundaries).
        def drain_one(g):
            # throttle injections: one deferred item per group keeps the
            # exp cadence close to back-to-back; groups that already took
            # a QKV filler skip work items unless the queue backs up
            took_filler = False
            if fillers:
                fillers.popleft()()
                took_filler = True
            if g < 2:
                return
            if took_filler and len(work_q) <= 8:
                return
            n = 2 if len(work_q) > 8 else min(1, len(work_q))
            for _ in range(min(n, len(work_q))):
                work_q.popleft()()

        def emit_proj(tb, full):
            def s1():
                b256, half = tb // 2, tb % 2
                col = _paired_col(b256) + 128 * half
                r1 = 128 if full else HD
                pos = []
                for c0, c1 in ((0, 512), (512, 768)):
                    po = ps.tile([128, c1 - c0], F32, name="po", tag="ps")
                    nc.tensor.matmul(
                        po,
                        lhsT=yt_all[0:r1, col : col + 128],
                        rhs=wp_sb[0:r1, c0:c1],
                        start=True,
                        stop=True,
                    )
                    pos.append(po)

                def s2():
                    osb = opool.tile([128, C], BF16, name="osb", tag="osb")
                    for (c0, c1), po in zip(((0, 512), (512, 768)), pos):
                        nc.vector.tensor_copy(out=osb[:, c0:c1], in_=po)
                    nc.sync.dma_start(
                        out=out_d.ap()[tb * 128 : (tb + 1) * 128, :], in_=osb
                    )

                work_q.append(s2)

            return s1

        def emit_norm(ytsb, r0, r1, col, after=()):
            """after: closures enqueued once this norm's mul has been
            emitted (used to order proj emission behind yt_all writes)."""

            def s1():
                r_sb = rpool.tile([1, 2 * QB], F32R, name="r_sb", tag="r_sb", bufs=8)
                with nc.allow_low_precision(reason="softmax denom broadcast"):
                    nc.vector.reciprocal(out=r_sb, in_=ytsb[HD : HD + 1, :])
                bc = ps.tile([HD, 2 * QB], F32, name="bc", tag="ps")
                nc.tensor.matmul(bc, lhsT=ones64, rhs=r_sb, start=True, stop=True)

                def s2():
                    nc.vector.tensor_mul(
                        out=yt_all[r0:r1, col : col + 2 * QB],
                        in0=ytsb[0:HD, :],
                        in1=bc,
                    )
                    work_q.extend(after)

                work_q.append(s2)

            return s1

        def emit_pair(s, i, after=()):
            """Attention for slot s (d-rows [64s, 64s+64)), pair i."""
            r0, r1 = s * HD, (s + 1) * HD
            qcol = 2 * QB * i
            n_shared = 2 * i + 2
            n_total = NKB - 2 * i
            diag_b0 = NKB - 2 - 2 * i  # first diagonal k-block of side B
            yt = ps_yt.tile([HD + 1, 2 * QB], F32, name="yt", tag="yt")
            groups = [(g, g + 2) for g in range(0, n_shared, 2)]
            kb0 = n_shared
            while kb0 < n_total:
                n = min(4, n_total - kb0)
                groups.append((kb0, kb0 + n))
                kb0 += n

            def emit_s(grp):
                ka, kb = grp
                shared = ka < n_shared
                w = 2 * QB if shared else QB
                qoff = qcol if shared else qcol + QB
                gw = w * (kb - ka)
                st = ps_st.tile([128, 4 * QB], F32, name="st", tag="st")
                for j in range(kb - ka):
                    blk = ka + j
                    nc.tensor.matmul(
                        st[:, j * w : (j + 1) * w],
                        lhsT=kt[r0:r1, blk * KB : (blk + 1) * KB],
                        rhs=qt[r0:r1, qoff : qoff + w],
                        start=True,
                        stop=True,
                    )
                # causal mask: -1e30 bias onto the diagonal blocks' own half
                for j in range(kb - ka):
                    blk = ka + j
                    if shared and blk in (2 * i, 2 * i + 1):
                        nc.vector.tensor_add(
                            out=st[:, j * w : j * w + QB],
                            in0=st[:, j * w : j * w + QB],
                            in1=mbias[:, blk - 2 * i, :],
                        )
                    elif not shared and blk in (diag_b0, diag_b0 + 1):
                        nc.vector.tensor_add(
                            out=st[:, j * w : (j + 1) * w],
                            in0=st[:, j * w : (j + 1) * w],
                            in1=mbias[:, blk - diag_b0, :],
                        )
                pt = ptpool.tile([128, 4 * QB], BF16, name="pt", tag="pt")
                nc.scalar.activation(
                    out=pt[:, 0:gw], in_=st[:, 0:gw], func=EXP, scale=scale
                )
                return pt, w

            def emit_pv(grp, pt, w):
                ka, kb = grp
                shared = ka < n_shared
                for j in range(kb - ka):
                    blk = ka + j
                    if shared:
                        nc.tensor.matmul(
                            yt,
                            lhsT=v1[:, blk, s, :],
                            rhs=pt[:, j * w : (j + 1) * w],
                            start=(blk == 0),
                            stop=False,
                            skip_group_check=True,
                        )
                    else:
                        nc.tensor.matmul(
                            yt[:, QB : 2 * QB],
                            lhsT=v1[:, blk, s, :],
                            rhs=pt[:, j * w : (j + 1) * w],
                            start=False,
                            stop=(blk == n_total - 1),
                            skip_group_check=True,
                        )

            pending = None
            for gi, grp in enumerate(groups):
                cur = (grp, *emit_s(grp))
                if pending is not None:
                    emit_pv(*pending)
                pending = cur
                drain_one(gi)
            emit_pv(*pending)
            # free the yt PSUM slot quickly; normalization is deferred
            ytsb = rpool.tile([HD + 1, 2 * QB], F32, name="ytsb", tag="ytsb", bufs=6)
            nc.vector.tensor_copy(out=ytsb, in_=yt)
            work_q.append(emit_norm(ytsb, r0, r1, qcol, after))

        # ---- schedule ----
        # Pair processing order: causal-need ascending at the front (pair 7
        # needs the least K/V), and a slot0-only pair LAST so the final
        # norm->proj tail is short and runs on a warm PE.
        order = [7, 6, 5, 3, 2, 1, 0, 4]

        # upfront: exactly what pair 7 needs at its start
        for ts in range(5):
            emit_k(ts)
        emit_q(3)
        emit_q(4)
        for tb in range(10):
            emit_v(tb)

        # remaining QKV units with due POSITION in the processing order
        # (first position whose pair consumes them; V for a pair's own
        # tail k-blocks gets one position of stream-in slack)
        due = {
            1: [lambda tb=tb: emit_v(tb) for tb in range(10, 18)],
            2: [lambda: emit_q(2), lambda: emit_q(5)]
            + [lambda tb=tb: emit_v(tb) for tb in range(18, 20)],
            3: [lambda: emit_k(5), lambda: emit_k(6), lambda: emit_q(1),
                lambda: emit_q(6)]
            + [lambda tb=tb: emit_v(tb) for tb in range(20, 24)],
            4: [lambda tb=tb: emit_v(tb) for tb in range(24, 26)],
            5: [lambda: emit_k(7), lambda: emit_q(0), lambda: emit_q(7)]
            + [lambda tb=tb: emit_v(tb) for tb in range(26, 28)],
            6: [lambda tb=tb: emit_v(tb) for tb in range(28, 30)],
            7: [lambda tb=tb: emit_v(tb) for tb in range(30, 32)],
        }
        for p in range(1, NPAIR):
            fillers.extend(due.get(p, []))
        # fillers allowed to remain when position p starts = units due later
        allowed = {
            p: sum(len(due.get(j, [])) for j in range(p + 1, NPAIR))
            for p in range(NPAIR)
        }

        for p, i in enumerate(order):
            while len(fillers) > allowed[p]:
                fillers.popleft()()
            full = i in s1_pairs
            projs = [
                emit_proj(tb, full)
                for tb in (2 * i, 2 * i + 1, NKB - 2 - 2 * i, NKB - 1 - 2 * i)
            ]
            emit_pair(0, i, after=() if full else projs)
            if full:
                emit_pair(1, i, after=projs)
        while fillers:
            fillers.popleft()()
        while work_q:
            work_q.popleft()()

    nc.compile()
    return nc


def _get_ncs():
    if "ncs" not in _CACHE:
        _CACHE["ncs"] = [_build_nc(0), _build_nc(1)]
    return _CACHE["ncs"]


def _core_inputs(x, w_attn, w_proj):
    """Build per-core input dicts (bf16, pre-transposed x, head slices)."""
    import ml_dtypes

    bf16 = ml_dtypes.bfloat16
    xt = np.ascontiguousarray(x.reshape(T, C).T.astype(bf16))
    w_attn = np.asarray(w_attn, dtype=np.float32)
    w_proj = np.asarray(w_proj, dtype=np.float32)
    in_maps = []
    for c in range(N_CORES):
        hF = c
        hH = 8 + (c % 4)
        wa = np.zeros((C, 3, 2, HD), dtype=np.float32)
        wp = np.zeros((2 * HD, C), dtype=np.float32)
        for s, h in enumerate((hF, hH)):
            for p in range(3):
                wa[:, p, s, :] = w_attn[:, p * C + h * HD : p * C + (h + 1) * HD]
            wp[s * HD : (s + 1) * HD, :] = w_proj[h * HD : (h + 1) * HD, :]
        in_maps.append(
            {
                "xt": xt,
                "wa": np.ascontiguousarray(wa.reshape(C, 3 * 2 * HD)).astype(bf16),
                "wp": wp.astype(bf16),
            }
        )
    return in_maps


def _make_sharded(nc, devices):
    """Build one 4-core shard_map'd PJRT executable for a program variant."""
    import jax
    import concourse.mybir as mybir
    from concourse import bass2jax
    from jax.experimental.shard_map import shard_map
    from jax.sharding import Mesh, PartitionSpec

    in_names, out_names, out_avals, zero_outs = [], [], [], []
    for alloc in nc.m.functions[0].allocations:
        if not isinstance(alloc, mybir.MemoryLocationSet):
            continue
        name = alloc.memorylocations[0].name
        if alloc.kind == "ExternalInput":
            if nc.partition_id_tensor and name == nc.partition_id_tensor.name:
                continue
            in_names.append(name)
        elif alloc.kind == "ExternalOutput":
            shape = tuple(alloc.tensor_shape)
            dtype = mybir.dt.np(alloc.dtype)
            out_names.append(name)
            out_avals.append(jax.core.ShapedArray(shape, dtype))
            zero_outs.append(np.zeros(shape, dtype))
    n_params = len(in_names)
    all_in_names = in_names + out_names
    if nc.partition_id_tensor:
        all_in_names = all_in_names + [nc.partition_id_tensor.name]

    def _body(*args):
        operands = list(args)
        if nc.partition_id_tensor:
            operands.append(bass2jax.partition_id_tensor())
        outs = bass2jax._bass_exec_p.bind(
            *operands,
            out_avals=tuple(out_avals),
            in_names=tuple(all_in_names),
            out_names=tuple(out_names),
            lowering_input_output_aliases=(),
            sim_require_finite=True,
            sim_require_nnan=True,
            nc=nc,
        )
        return tuple(outs)

    mesh = Mesh(np.asarray(devices), ("core",))
    n_out = len(out_names)
    donate = tuple(range(n_params, n_params + n_out))
    sharded = jax.jit(
        shard_map(
            _body,
            mesh=mesh,
            in_specs=(PartitionSpec("core"),) * (n_params + n_out),
            out_specs=(PartitionSpec("core"),) * n_out,
            check_rep=False,
        ),
        donate_argnums=donate,
        keep_unused=True,
    )
    return sharded, in_names, out_names, out_avals, zero_outs


def _get_runner():
    if "runner" in _CACHE:
        return _CACHE["runner"]
    import jax
    from concourse import bass2jax

    ncs = _get_ncs()
    bass2jax.install_neuronx_cc_hook()
    devices = jax.devices()[:N_CORES]
    execs = [
        _make_sharded(ncs[0], devices[0:4]),
        _make_sharded(ncs[1], devices[4:8]),
    ]

    def run(in_maps):
        results = [None] * N_CORES
        pending = []
        for v, (sharded, in_names, out_names, out_avals, zero_outs) in enumerate(
            execs
        ):
            cores = range(4 * v, 4 * v + 4)
            concat_in = [
                np.concatenate([in_maps[c][name] for c in cores], axis=0)
                for name in in_names
            ]
            concat_zeros = [
                np.zeros((4 * z.shape[0], *z.shape[1:]), z.dtype) for z in zero_outs
            ]
            out_arrs = sharded(*concat_in, *concat_zeros)
            pending.append((v, out_names, out_avals, out_arrs))
        for v, out_names, out_avals, out_arrs in pending:
            for i, name in enumerate(out_names):
                arr = np.asarray(out_arrs[i]).reshape(4, *out_avals[i].shape)
                for j in range(4):
                    c = 4 * v + j
                    if results[c] is None:
                        results[c] = {}
                    results[c][name] = arr[j]
        return results

    _CACHE["runner"] = run
    return run


def kernel(x, w_attn, w_proj):
    run = _get_runner()
    in_maps = _core_inputs(np.asarray(x), np.asarray(w_attn), np.asarray(w_proj))
    results = run(in_maps)
    out = np.zeros((T, C), dtype=np.float32)
    for c in range(N_CORES):
        out += results[c]["out"].astype(np.float32)
    return out.reshape(1, T, C)
